# revision 15
# baseline (speedup 1.0000x reference)
# Trainium2 Bass kernel for nn_LocalAggregator (Gaussian -> voxel-grid semantic
# compositing).
#
# Strategy: the voxel grid (60,60,36) is tiled into 1080 3-D blocks of
# (4,5,6) = 120 voxels.  Blocks are dealt to the 8 cores by sorted hit-count
# (rank 8k+c -> core c, slot k) so every core sees a near-identical load
# profile; the host un-permutes the output afterwards.  For each block the
# host builds the exact list of Gaussians whose integer box overlaps the
# block in all three dims.  In block-local coordinates the feature matrix
# (quadratic monomials + one-hot rows for the x/y/z box tests) is the SAME
# for every block, so a single small RHS is shared by all matmuls; all
# per-(block,Gaussian) data lives in the COEF matrix.  The E matmul runs in
# bf16 with a 3-way hi/lo product split (rows [hi,hi,lo] x features
# [fhi,flo,fhi] drop only the lo*lo term, ~1e-5 relative), Exp runs on the
# scalar engine over 12 units (3 PSUM banks x 4 blocks each) at a time, and
# the semantic reduction is an fp16 matmul accumulated into PSUM banks packed
# 12 blocks deep (3 partition stripes x 4 column slots) so one DVE copy +
# one Pool-engine DMA drains 12 blocks at once.  Dummy matmuls at t=0 ramp
# the PE clock to max p-state and a dummy activation preloads the Exp table.
import numpy as np
import ml_dtypes

H, W, D = 60, 60, 36
GRID = 0.08
SCALE_MULT = 3.0
P = 2048
C = 13
N = H * W * D                  # 129600
NCORES = 8
BX, BY, BZ = 4, 5, 6           # block shape
NBX, NBY, NBZ = H // BX, W // BY, D // BZ
NB = NBX * NBY * NBZ           # 1080 blocks total
GPTS = BX * BY * BZ            # 120 points per block
NSLOT = NB // NCORES           # 135 blocks (slots) per core
NPC = NSLOT * GPTS             # 16200 points per core
KF = 30 + BX + BY + BZ         # 45 feature rows (3x10 split products + onehots)
PEN = -2000.0                  # box-miss penalty (exp() == 0 in fp32)
PERBANK = 4                    # 120-col units per 512-f32 PSUM bank
NBANK = 2                      # banks per psE tile / act instruction
UPT = PERBANK * NBANK          # 8 cslots per psE tile
OGRP = 12                      # blocks per psO bank (3 stripes x 4 col slots)
OROWS = 77                     # 2*32+13 live partitions per output group
OCOLS = PERBANK * GPTS         # 480 cols per output group
NGRP = (NSLOT + OGRP - 1) // OGRP   # output groups per core (last partial)

_NC_CACHE: dict = {}
_JIT_CACHE: dict = {}


def _pack_units(L_slots):
    """Bin-pack units into 128-partition column-slots.

    All E matmuls share one RHS, so any units whose COEF columns are laid
    out adjacently can share a single E matmul (M = sum Mt <= 128) and one
    activation column range.  The per-block semantic matmul then contracts
    K = bin rows from base partition 0; zero semantics rows mask the other
    blocks in the bin.  Returns (cslots, grp_of, gg_of, gsz_of, Ltot) with
    cslots a list of bins, each a list of
    (slot, coef_off, Mt, row_off, first_in_slot, last_in_slot, s0).
    psO groups follow the slot completion order; bins are ordered to keep
    the per-activation-tile unit count flat.
    """
    raw = []
    for g, L in enumerate(L_slots):
        L = int(L)
        for s in range(0, L, 128):
            raw.append((g, min(128, L - s), s))       # (slot, Mt, s0)
    # psO groups in completion order: OGRP-slot groups, plus a tiny final
    # group so the very last drain's copy+DMA chain is short.  Packing
    # regions = consecutive group PAIRS (psO bufs = 2), so a region's bins
    # never reference a psO tile that hasn't been freed by an earlier drain.
    VN_ = len(L_slots)
    if VN_ <= 3:
        gsizes = [VN_]
    else:
        r2 = 2
        k, rem1 = divmod(VN_ - r2, OGRP)
        gsizes = [OGRP] * k + ([rem1] if rem1 else []) + [r2]
    regions = [gsizes[i] + (gsizes[i + 1] if i + 1 < len(gsizes) else 0)
               for i in range(0, len(gsizes), 2)]
    seq = []
    r0 = 0
    for rs in regions:
        idxs = [i for i in range(len(raw)) if r0 <= raw[i][0] < r0 + rs]
        idxs.sort(key=lambda i: -raw[i][1])
        bins = []
        for i in idxs:
            Mt = raw[i][1]
            for b in bins:
                if b[0] + Mt <= 128:
                    b[1].append(i)
                    b[0] += Mt
                    break
            else:
                bins.append([Mt, [i]])
        seq.extend(bins)
        r0 += rs
    # coef offsets in processing order + row offsets within each bin
    cslots = []
    cum = 0
    placed = {}
    for b in seq:
        out = []
        ro = 0
        for i in b[1]:
            g, Mt, s0 = raw[i]
            out.append([g, cum, Mt, ro, False, False, s0])
            placed[i] = (len(cslots), len(out) - 1)
            cum += Mt
            ro += Mt
        cslots.append(out)
    Ltot = cum
    # first/last flags per slot by processing (cslot) order
    by_slot = {}
    for ci, cs in enumerate(cslots):
        for ji, u in enumerate(cs):
            by_slot.setdefault(u[0], []).append((ci, ji))
    for g, lst in by_slot.items():
        lst.sort()
        ci, ji = lst[0]
        cslots[ci][ji][4] = True
        ci, ji = lst[-1]
        cslots[ci][ji][5] = True
    cslots = [[tuple(u) + (False,) for u in cs] for cs in cslots]
    # slot completion order -> psO groups
    comp = {}
    for ci, cs in enumerate(cslots):
        for u in cs:
            comp[u[0]] = max(comp.get(u[0], -1), ci)
    order = sorted(range(len(L_slots)), key=lambda g: (comp[g], g))
    grp_of = [0] * len(L_slots)
    gg_of = [0] * len(L_slots)
    gsz_of = [0] * len(L_slots)
    gstart = [0]
    for sz in gsizes:
        gstart.append(gstart[-1] + sz)
    for pos, g in enumerate(order):
        q = 0
        while gstart[q + 1] <= pos:
            q += 1
        grp_of[g] = q
        gg_of[g] = pos - gstart[q]
        gsz_of[g] = gsizes[q]
    return cslots, grp_of, gg_of, gsz_of, gsizes, Ltot


def _build_nc(L_slots):
    import concourse.bacc as bacc
    import concourse.tile as tile
    from concourse import mybir

    L_slots = [int(x) for x in L_slots]
    VN = len(L_slots)
    cslots, grp_of, gg_of, gsz_of, gsizes, Ltot = _pack_units(L_slots)
    NG = len(gsizes)
    units = [u for cs in cslots for u in cs]
    NU = len(units)
    NCS = len(cslots)
    # first tile is a single bank (PERBANK cslots) so the first activation
    # fires as soon as possible after the first COEF chunk lands
    tb = [0, min(PERBANK, NCS)]
    while tb[-1] < NCS:
        tb.append(min(tb[-1] + UPT, NCS))
    tiles_u = [(tb[i], cslots[tb[i]:tb[i + 1]]) for i in range(len(tb) - 1)]

    nc = bacc.Bacc("TRN2", target_bir_lowering=False, debug=False,
                   num_devices=NCORES)
    f32 = mybir.dt.float32
    bf16 = mybir.dt.bfloat16
    f16 = mybir.dt.float16
    COEF = nc.dram_tensor("COEF", [KF, GPTS + Ltot], bf16,
                          kind="ExternalInput")
    SEMP = nc.dram_tensor("SEMP", [128, NU * C], f16, kind="ExternalInput")
    OUT = nc.dram_tensor("OUT", [OROWS, NG * OCOLS], f16,
                         kind="ExternalOutput")

    usem = {}
    for ui_, u_ in enumerate(units):
        usem[(u_[0], u_[1])] = ui_

    # drain each psO group right after its last-emitted unit
    last_pos = {}
    for ci_, cs_ in enumerate(cslots):
        for ji_, u_ in enumerate(cs_):
            last_pos[grp_of[u_[0]]] = (ci_, ji_)
    drain_at = {}
    for grp_, pos_ in last_pos.items():
        drain_at.setdefault(pos_, []).append(grp_)

    # coef-chunk boundaries at cslot granularity (processing order):
    # chunk 0 = first tile (HWDGE, lands first), chunk 1 = next ~2 tiles
    # (SWDGE on the idle Pool engine, generated in parallel), chunks 2/3 =
    # the rest split in two (HWDGE) so no tile waits on one huge transfer.
    def _coff(ci):
        return cslots[ci][0][1] if ci < NCS else Ltot
    c1 = min(PERBANK, NCS)
    c2 = min(c1 + 2 * UPT, NCS)
    c3 = min(c2 + 4 * UPT, NCS)
    cb = [0, _coff(c1), _coff(c2), _coff(c3), Ltot]
    # semp split: first half covers units used by the first ~5 tiles
    cs_lens = [len(cs) for cs in cslots]
    u_s0 = sum(cs_lens[:min(5 * UPT, NCS)])
    u_s0 = max(1, min(NU, u_s0))

    with tile.TileContext(nc) as tc:
        with (
            tc.tile_pool(name="big", bufs=1) as big_pool,
            tc.tile_pool(name="w", bufs=6) as w_pool,
            tc.tile_pool(name="og", bufs=4) as og_pool,
            tc.tile_pool(name="psE", bufs=3, space="PSUM") as pse_pool,
            tc.tile_pool(name="psO", bufs=2, space="PSUM") as pso_pool,
        ):
            coefx_b = big_pool.tile([KF, GPTS + Ltot], bf16)
            rhs_b = coefx_b[:, 0:GPTS]
            coef_b = coefx_b[:, GPTS:]
            semp_b = big_pool.tile([128, NU * C], f16)
            scr_b = big_pool.tile([1, 512], bf16)
            scr_o = big_pool.tile([1, 8], f16)

            # --- warmup: preload the Exp activation table while inputs load
            nc.tensor.write(scr_b[0:1, 0:1],
                            np.zeros(1, ml_dtypes.bfloat16).tobytes())
            nc.scalar.activation(scr_o[0:1, 0:1], scr_b[0:1, 0:1],
                                 mybir.ActivationFunctionType.Exp)

            # --- input loads: chunk0 on SP HWDGE (first to land), tiles 1-2
            # + late semp on Pool SWDGE (parallel generation), rest on SP.
            nc.sync.dma_start(coefx_b[:, 0:GPTS + cb[1]],
                              COEF[:, 0:GPTS + cb[1]])
            if cb[1] < cb[2]:
                nc.gpsimd.dma_start(coef_b[:, cb[1]:cb[2]],
                                    COEF[:, GPTS + cb[1]:GPTS + cb[2]])
            nc.sync.dma_start(semp_b[:, 0:u_s0 * C], SEMP[:, 0:u_s0 * C])
            if u_s0 < NU:
                nc.gpsimd.dma_start(semp_b[:, u_s0 * C:], SEMP[:, u_s0 * C:])
            if cb[2] < cb[3]:
                nc.sync.dma_start(coef_b[:, cb[2]:cb[3]],
                                  COEF[:, GPTS + cb[2]:GPTS + cb[3]])
            if cb[3] < Ltot:
                nc.sync.dma_start(coef_b[:, cb[3]:cb[4]],
                                  COEF[:, GPTS + cb[3]:GPTS + cb[4]])

            pso_t = {}    # group -> psO tile
            w_ts, psEs = {}, {}

            def emit_E(t):
                toff, tu = tiles_u[t]
                psE = pse_pool.tile([128, NBANK, 512], f32, name=f"psE{t}",
                                    tag="psE")
                psEs[t] = psE
                for j, cs in enumerate(tu):
                    b, s = j // PERBANK, j % PERBANK
                    off0 = cs[0][1]
                    rows = sum(u[2] for u in cs)
                    nc.tensor.matmul(
                        psE[0:rows, b:b + 1, s * GPTS:(s + 1) * GPTS],
                        coef_b[:, off0:off0 + rows], rhs_b[:],
                        start=True, stop=True, skip_group_check=True)

            def emit_act(t):
                toff, tu = tiles_u[t]
                psE = psEs[t]
                w_t = w_pool.tile([128, NBANK, OCOLS], f16, name=f"w{t}",
                                  tag="w")
                w_ts[t] = w_t
                mtmax = max(sum(u[2] for u in cs) for cs in tu)
                nb_full, rem = divmod(len(tu), PERBANK)
                if nb_full:
                    nc.scalar.activation(
                        w_t[0:mtmax, 0:nb_full, :],
                        psE[0:mtmax, 0:nb_full, 0:OCOLS],
                        mybir.ActivationFunctionType.Exp)
                if rem:
                    nc.scalar.activation(
                        w_t[0:mtmax, nb_full:nb_full + 1, 0:rem * GPTS],
                        psE[0:mtmax, nb_full:nb_full + 1, 0:rem * GPTS],
                        mybir.ActivationFunctionType.Exp)

            def emit_out(t):
                toff, tu = tiles_u[t]
                w_t = w_ts[t]
                for j, cs in enumerate(tu):
                  b, js = j // PERBANK, j % PERBANK
                  rows = sum(u[2] for u in cs)
                  for ji, (g, off, Mt, po, first, last, s0, pd) \
                          in enumerate(cs):
                    grp, gg, gsz = grp_of[g], gg_of[g], gsz_of[g]
                    gi, s = gg // PERBANK, gg % PERBANK
                    ui = usem[(g, off)]
                    if grp not in pso_t:
                        pso_t[grp] = pso_pool.tile([OROWS, OCOLS], f32,
                                                   name=f"psO{grp}",
                                                   tag="psO")
                    nc.tensor.matmul(
                        pso_t[grp][gi * 32:gi * 32 + C,
                                   s * GPTS:(s + 1) * GPTS],
                        semp_b[0:rows, ui * C:(ui + 1) * C],
                        w_t[0:rows, b:b + 1,
                            js * GPTS:(js + 1) * GPTS],
                        start=first, stop=last, skip_group_check=True)
                    for grp_d in drain_at.get((toff + j, ji), []):
                        gsz_d = gsizes[grp_d]
                        top = (gsz_d - 1) // PERBANK   # last stripe index
                        grows = top * 32 + C
                        gcols = OCOLS if top > 0 else gsz_d * GPTS
                        outg = og_pool.tile([OROWS, OCOLS], f16,
                                            name=f"og{grp_d}", tag="og")
                        nc.vector.tensor_copy(
                            outg[0:grows, 0:gcols],
                            pso_t[grp_d][0:grows, 0:gcols])
                        nc.sync.dma_start(
                            OUT[0:grows,
                                grp_d * OCOLS:grp_d * OCOLS + gcols],
                            outg[0:grows, 0:gcols])

            emit_E(0)
            for t in range(len(tiles_u)):
                emit_act(t)
                if t + 1 < len(tiles_u):
                    emit_E(t + 1)
                emit_out(t)
    nc.compile()
    return nc


def _get_nc(L_slots):
    key = tuple(int(x) for x in L_slots)
    if key not in _NC_CACHE:
        _NC_CACHE[key] = _build_nc(L_slots)
    return _NC_CACHE[key]


def _get_runner(nc):
    """Cached shard_map-jitted executor for one Bass program (axon/PJRT path).

    Mirrors concourse.bass2jax.run_bass_via_pjrt but keeps the jitted callable
    so repeated runs don't rebuild/recompile."""
    if id(nc) in _JIT_CACHE:
        return _JIT_CACHE[id(nc)]
    import jax
    from concourse import bass2jax, mybir
    from jax.experimental.shard_map import shard_map
    from jax.sharding import Mesh, PartitionSpec

    bass2jax.install_neuronx_cc_hook()
    partition_name = (nc.partition_id_tensor.name
                      if nc.partition_id_tensor else None)
    in_names, out_names, out_avals, zero_outs = [], [], [], []
    for alloc in nc.m.functions[0].allocations:
        if not isinstance(alloc, mybir.MemoryLocationSet):
            continue
        name = alloc.memorylocations[0].name
        if alloc.kind == "ExternalInput":
            if name == partition_name:
                continue
            in_names.append(name)
        elif alloc.kind == "ExternalOutput":
            shape = tuple(alloc.tensor_shape)
            dtype = mybir.dt.np(alloc.dtype)
            out_names.append(name)
            out_avals.append(jax.core.ShapedArray(shape, dtype))
            zero_outs.append(np.zeros(shape, dtype))
    n_params = len(in_names)
    all_in_names = in_names + out_names
    if partition_name is not None:
        all_in_names = all_in_names + [partition_name]

    def _body(*args):
        operands = list(args)
        if partition_name is not None:
            operands.append(bass2jax.partition_id_tensor())
        outs = bass2jax._bass_exec_p.bind(
            *operands,
            out_avals=tuple(out_avals),
            in_names=tuple(all_in_names),
            out_names=tuple(out_names),
            lowering_input_output_aliases=(),
            sim_require_finite=True,
            sim_require_nnan=True,
            nc=nc,
        )
        return tuple(outs)

    devices = jax.devices()[:NCORES]
    mesh = Mesh(np.asarray(devices), ("core",))
    donate = tuple(range(n_params, n_params + len(out_names)))
    sharded = jax.jit(
        shard_map(_body, mesh=mesh,
                  in_specs=(PartitionSpec("core"),) * (n_params + len(out_names)),
                  out_specs=(PartitionSpec("core"),) * len(out_names),
                  check_rep=False),
        donate_argnums=donate, keep_unused=True)

    def run(in_maps, rounds=1):
        concat_in = [np.concatenate([np.asarray(m[nm]) for m in in_maps], axis=0)
                     for nm in in_names]
        outs = None
        for _ in range(rounds):
            zo = [np.concatenate([z] * NCORES, axis=0) for z in zero_outs]
            outs = sharded(*concat_in, *zo)
        outs = [np.asarray(o) for o in outs]
        results = []
        for ci in range(NCORES):
            d = {}
            for oi, nm in enumerate(out_names):
                per = outs[oi].shape[0] // NCORES
                d[nm] = outs[oi][ci * per:(ci + 1) * per]
            results.append(d)
        return results, sharded, (concat_in, zero_outs, in_names, out_names)

    sharded_nd = jax.jit(
        shard_map(_body, mesh=mesh,
                  in_specs=(PartitionSpec("core"),) * (n_params + len(out_names)),
                  out_specs=(PartitionSpec("core"),) * len(out_names),
                  check_rep=False),
        keep_unused=True)

    def timeit(in_maps, iters=30):
        import time as _time
        from jax.sharding import NamedSharding
        sh = NamedSharding(mesh, PartitionSpec("core"))
        concat_in = [np.concatenate([np.asarray(m[nm]) for m in in_maps], axis=0)
                     for nm in in_names]
        zo = [np.concatenate([z] * NCORES, axis=0) for z in zero_outs]
        args = [jax.device_put(a, sh) for a in concat_in + zo]
        outs = sharded_nd(*args)
        jax.block_until_ready(outs)
        t0 = _time.time()
        for _ in range(iters):
            outs = sharded_nd(*args)
        jax.block_until_ready(outs)
        return (_time.time() - t0) / iters

    run.timeit = timeit
    _JIT_CACHE[id(nc)] = run
    return run


def _bf16_split(x):
    hi = x.astype(ml_dtypes.bfloat16).astype(np.float32)
    return hi, (x - hi).astype(np.float32)


def _host_prep(pts, means3D, opacities, semantics, scales, cov3D, origin_use):
    pts = np.asarray(pts, np.float32).reshape(N, 3)
    mu32 = np.asarray(means3D, np.float32).reshape(P, 3)
    op = np.asarray(opacities, np.float64).reshape(P)
    sem = np.asarray(semantics, np.float32).reshape(P, C)
    sc32 = np.asarray(scales, np.float32).reshape(P, 3)
    cov = np.asarray(cov3D, np.float64).reshape(P, 3, 3)
    org32 = np.asarray(origin_use, np.float32).reshape(3)

    # --- integer binning, replicated in fp32 exactly like the reference ---
    radii = np.ceil(sc32.max(-1) * np.float32(SCALE_MULT) / np.float32(GRID)
                    ).astype(np.int32).astype(np.int64)
    m_int = ((mu32 - org32) / np.float32(GRID)).astype(np.int32).astype(np.int64)
    p_int = ((pts - org32) / np.float32(GRID)).astype(np.int32).astype(np.int64)

    # structured-input check: points must be the (i, j, k) voxel-center grid
    idx = np.arange(N)
    kk = idx % D
    col = idx // D
    jj = col % W
    ii = col // W
    grid_int = np.stack([ii, jj, kk], axis=-1)
    if not np.array_equal(p_int, grid_int):
        raise RuntimeError("kernel: unstructured pts not supported by fast path")

    # --- per-Gaussian inverse covariance (float64) ---
    a, b, c_, d, e, f = (cov[:, 0, 0], cov[:, 1, 1], cov[:, 2, 2],
                         cov[:, 0, 1], cov[:, 1, 2], cov[:, 0, 2])
    det = a * (b * c_ - e * e) - d * (d * c_ - e * f) + f * (d * e - b * f)
    ixx = (b * c_ - e * e) / det
    iyy = (a * c_ - f * f) / det
    izz = (a * b - d * d) / det
    ixy = (e * f - d * c_) / det
    iyz = (d * f - a * e) / det
    ixz = (d * e - b * f) / det
    A = np.empty((P, 3, 3))
    A[:, 0, 0], A[:, 1, 1], A[:, 2, 2] = ixx, iyy, izz
    A[:, 0, 1] = A[:, 1, 0] = ixy
    A[:, 1, 2] = A[:, 2, 1] = iyz
    A[:, 0, 2] = A[:, 2, 0] = ixz
    mu = mu32.astype(np.float64)
    logop = np.log(op)

    # --- blocks: bounds, hit lists, load-balanced assignment ---
    mx, my, mz = m_int[:, 0], m_int[:, 1], m_int[:, 2]
    blocks = []            # (x0, y0, z0)
    hits = []
    for bxi in range(NBX):
        for byi in range(NBY):
            for bzi in range(NBZ):
                x0, y0, z0 = bxi * BX, byi * BY, bzi * BZ
                hit = np.where(
                    (mx + radii >= x0) & (mx - radii <= x0 + BX - 1) &
                    (my + radii >= y0) & (my - radii <= y0 + BY - 1) &
                    (mz + radii >= z0) & (mz - radii <= z0 + BZ - 1))[0]
                blocks.append((x0, y0, z0))
                hits.append(hit)
    Ls = np.array([len(h) for h in hits])
    order = np.argsort(-Ls, kind="stable")     # rank r -> block id
    # slot k <- rank seq_ranks[k]: deal ranks from thirds so every
    # region of slots mixes large/mid/small hit lists
    t3 = (NSLOT + 2) // 3
    seq_ranks = []
    for i in range(t3):
        for base in (0, t3, 2 * t3):
            if base + i < NSLOT:
                seq_ranks.append(base + i)
    # core c, slot k gets block order[8*seq_ranks[k] + c].  Slots with
    # more than 128 hits are split into independent virtual slots so
    # every psO accumulation is a single start+stop matmul.
    raw_L = [max(1, int(Ls[order[8 * seq_ranks[k]]])) for k in range(NSLOT)]
    L_slots, vmap = [], []
    for k, L in enumerate(raw_L):
        L_slots.append(min(128, L))
        vmap.append((k, 0))
        if L > 128:
            L_slots.append(L - 128)
            vmap.append((k, 128))
    VN = len(L_slots)
    cslots, grp_of, gg_of, gsz_of, gsizes, Ltot = _pack_units(L_slots)
    units = [u for cs in cslots for u in cs]
    NU = len(units)

    # --- shared RHS: block-local features (hi/lo pairs) + one-hot rows ---
    lx = np.arange(GPTS) // (BY * BZ)
    ly = (np.arange(GPTS) // BZ) % BY
    lz = np.arange(GPTS) % BZ
    xi = ((lx - (BX - 1) / 2.0) * GRID).astype(np.float32)
    yi = ((ly - (BY - 1) / 2.0) * GRID).astype(np.float32)
    zi = ((lz - (BZ - 1) / 2.0) * GRID).astype(np.float32)
    feat10 = np.stack([xi * xi, yi * yi, zi * zi, xi * yi, yi * zi, xi * zi,
                       xi, yi, zi, np.ones(GPTS, np.float32)])
    fhi, flo = _bf16_split(feat10)
    rhs = np.zeros((KF, GPTS), np.float32)
    rhs[0:10] = fhi
    rhs[10:20] = flo
    rhs[20:30] = fhi
    rhs[30 + lx, np.arange(GPTS)] = 1.0
    rhs[30 + BX + ly, np.arange(GPTS)] = 1.0
    rhs[30 + BX + BY + lz, np.arange(GPTS)] = 1.0
    rhs = rhs.astype(ml_dtypes.bfloat16)

    # --- per-core COEF / SEMP ---
    in_maps = []
    perm_blocks = []       # per core: slot -> block id
    # per-slot units (in processing order they appear)
    units_of_slot = {}
    for u in units:
        units_of_slot.setdefault(u[0], []).append(u)
    for ci in range(NCORES):
        coef_m = np.zeros((KF, Ltot), np.float32)
        semp = np.zeros((128, NU * C), np.float16)
        my_blocks = []
        uindex = {}
        for ui, u in enumerate(units):
            uindex[(u[0], u[1])] = ui
        for g in range(VN):
            k, vs0 = vmap[g]
            bid = int(order[8 * seq_ranks[k] + ci])
            my_blocks.append((bid, vs0))
            hit = hits[bid]
            nh = len(hit)
            if nh == 0 or vs0 >= nh:
                continue
            x0, y0, z0 = blocks[bid]
            cx = (x0 + (BX - 1) / 2.0 + 0.5) * GRID
            cy = (y0 + (BY - 1) / 2.0 + 0.5) * GRID
            cz = (z0 + (BZ - 1) / 2.0 + 0.5) * GRID
            dmu = mu[hit] - np.array([cx, cy, cz])      # [nh, 3]
            Ah = A[hit]                                  # [nh, 3, 3]
            Amu = np.einsum('pij,pj->pi', Ah, dmu)
            muAmu = np.einsum('pi,pi->p', dmu, Amu)
            c10 = np.stack([
                -0.5 * Ah[:, 0, 0], -0.5 * Ah[:, 1, 1], -0.5 * Ah[:, 2, 2],
                -Ah[:, 0, 1], -Ah[:, 1, 2], -Ah[:, 0, 2],
                Amu[:, 0], Amu[:, 1], Amu[:, 2],
                -0.5 * muAmu + logop[hit]]).astype(np.float32)
            chi, clo = _bf16_split(c10)
            in_x = (np.abs(x0 + np.arange(BX)[None, :] - mx[hit, None])
                    <= radii[hit, None])
            in_y = (np.abs(y0 + np.arange(BY)[None, :] - my[hit, None])
                    <= radii[hit, None])
            in_z = (np.abs(z0 + np.arange(BZ)[None, :] - mz[hit, None])
                    <= radii[hit, None])
            px = np.where(in_x, 0.0, PEN).T
            py = np.where(in_y, 0.0, PEN).T
            pz = np.where(in_z, 0.0, PEN).T
            semh = sem[hit].astype(np.float16)
            for (gg_, off, Mt, po, first, last, s0, pd) in units_of_slot[g]:
                a, b2 = vs0 + s0, min(vs0 + s0 + Mt, nh)
                if a >= nh:
                    continue
                w_ = b2 - a
                coef_m[0:10, off:off + w_] = chi[:, a:b2]
                coef_m[10:20, off:off + w_] = chi[:, a:b2]
                coef_m[20:30, off:off + w_] = clo[:, a:b2]
                coef_m[30:30 + BX, off:off + w_] = px[:, a:b2]
                coef_m[30 + BX:30 + BX + BY, off:off + w_] = py[:, a:b2]
                coef_m[30 + BX + BY:, off:off + w_] = pz[:, a:b2]
                ui = uindex[(gg_, off)]
                semp[po:po + w_, ui * C:(ui + 1) * C] = semh[a:b2]
        perm_blocks.append(my_blocks)
        coefx = np.concatenate([np.asarray(rhs, np.float32),
                                coef_m], axis=1).astype(ml_dtypes.bfloat16)
        in_maps.append({"COEF": coefx, "SEMP": semp})
    return in_maps, L_slots, (perm_blocks, grp_of, gg_of)


def kernel(**inputs):
    in_maps, L_slots, (perm_blocks, grp_of, gg_of) = _host_prep(**inputs)
    nc = _get_nc(L_slots)
    run = _get_runner(nc)
    results, _, _ = run(in_maps)
    out = np.empty((N, C), np.float32)
    lx = np.arange(GPTS) // (BY * BZ)
    ly = (np.arange(GPTS) // BZ) % BY
    lz = np.arange(GPTS) % BZ
    for ci in range(NCORES):
        o = results[ci]["OUT"].astype(np.float32)
        for g in range(len(perm_blocks[ci])):
            bid, vs0 = perm_blocks[ci][g]
            x0 = (bid // (NBY * NBZ)) * BX
            y0 = ((bid // NBZ) % NBY) * BY
            z0 = (bid % NBZ) * BZ
            grp, gg = grp_of[g], gg_of[g]
            gi, s = gg // PERBANK, gg % PERBANK
            col0 = grp * OCOLS + s * GPTS
            blk = o[gi * 32:gi * 32 + C, col0:col0 + GPTS]   # [C, GPTS]
            gidx = ((x0 + lx) * W + (y0 + ly)) * D + (z0 + lz)
            if vs0 == 0:
                out[gidx] = blk.T
            else:
                out[gidx] += blk.T
    return out



# revision 18
# speedup vs baseline: 1.0168x; 1.0168x over previous
# Trainium2 Bass kernel for nn_LocalAggregator (Gaussian -> voxel-grid semantic
# compositing).
#
# Strategy: the voxel grid (60,60,36) is tiled into 1080 3-D blocks of
# (4,5,6) = 120 voxels.  Blocks are dealt to the 8 cores by sorted hit-count
# (rank 8k+c -> core c, slot k) so every core sees a near-identical load
# profile; the host un-permutes the output afterwards.  For each block the
# host builds the exact list of Gaussians whose integer box overlaps the
# block in all three dims.  In block-local coordinates the feature matrix
# (quadratic monomials + one-hot rows for the x/y/z box tests) is the SAME
# for every block, so a single small RHS is shared by all matmuls; all
# per-(block,Gaussian) data lives in the COEF matrix.  The E matmul runs in
# bf16 with a 3-way hi/lo product split (rows [hi,hi,lo] x features
# [fhi,flo,fhi] drop only the lo*lo term, ~1e-5 relative), Exp runs on the
# scalar engine over 12 units (3 PSUM banks x 4 blocks each) at a time, and
# the semantic reduction is an fp16 matmul accumulated into PSUM banks packed
# 12 blocks deep (3 partition stripes x 4 column slots) so one DVE copy +
# one Pool-engine DMA drains 12 blocks at once.  Dummy matmuls at t=0 ramp
# the PE clock to max p-state and a dummy activation preloads the Exp table.
import numpy as np
import ml_dtypes

H, W, D = 60, 60, 36
GRID = 0.08
SCALE_MULT = 3.0
P = 2048
C = 13
N = H * W * D                  # 129600
NCORES = 8
BX, BY, BZ = 4, 5, 6           # block shape
NBX, NBY, NBZ = H // BX, W // BY, D // BZ
NB = NBX * NBY * NBZ           # 1080 blocks total
GPTS = BX * BY * BZ            # 120 points per block
NSLOT = NB // NCORES           # 135 blocks (slots) per core
NPC = NSLOT * GPTS             # 16200 points per core
KF = 30 + BX + BY + BZ         # 45 feature rows (3x10 split products + onehots)
PEN = -2000.0                  # box-miss penalty (exp() == 0 in fp32)
PERBANK = 4                    # 120-col units per 512-f32 PSUM bank
NBANK = 2                      # banks per psE tile / act instruction
UPT = PERBANK * NBANK          # 8 cslots per psE tile
OGRP = 12                      # blocks per psO bank (3 stripes x 4 col slots)
OROWS = 77                     # 2*32+13 live partitions per output group
OCOLS = PERBANK * GPTS         # 480 cols per output group
NGRP = (NSLOT + OGRP - 1) // OGRP   # output groups per core (last partial)

_NC_CACHE: dict = {}
_JIT_CACHE: dict = {}


def _pack_units(L_slots):
    """Bin-pack units into 128-partition column-slots.

    All E matmuls share one RHS, so any units whose COEF columns are laid
    out adjacently can share a single E matmul (M = sum Mt <= 128) and one
    activation column range.  The per-block semantic matmul then contracts
    K = bin rows from base partition 0; zero semantics rows mask the other
    blocks in the bin.  Returns (cslots, grp_of, gg_of, gsz_of, Ltot) with
    cslots a list of bins, each a list of
    (slot, coef_off, Mt, row_off, first_in_slot, last_in_slot, s0).
    psO groups follow the slot completion order; bins are ordered to keep
    the per-activation-tile unit count flat.
    """
    raw = []
    for g, L in enumerate(L_slots):
        L = int(L)
        for s in range(0, L, 128):
            raw.append((g, min(128, L - s), s))       # (slot, Mt, s0)
    # psO groups in completion order: OGRP-slot groups, plus a tiny final
    # group so the very last drain's copy+DMA chain is short.  Packing
    # regions = consecutive group PAIRS (psO bufs = 2), so a region's bins
    # never reference a psO tile that hasn't been freed by an earlier drain.
    VN_ = len(L_slots)
    k, rem1 = divmod(VN_, OGRP)
    if k == 0:
        gsizes = [rem1]
    elif rem1:
        gsizes = [OGRP] * k + [rem1]
    else:
        gsizes = [OGRP] * (k - 1) + [OGRP - 4, 4]
    # packing regions: pairs of groups, except the final two groups pack
    # SEQUENTIALLY (all of group NG-2's units strictly before group NG-1's)
    # so only the small final group's drain sits on the tail critical path
    regions = []          # (n_slots, sequential_subphases)
    i = 0
    while i < len(gsizes):
        if i + 2 >= len(gsizes):
            regions.append((sum(gsizes[i:]), gsizes[i:]))
            break
        regions.append((gsizes[i] + gsizes[i + 1], None))
        i += 2
    seq = []
    r0 = 0
    for rs, phases in regions:
        for ps in (phases or [rs]):
            idxs = [i for i in range(len(raw))
                    if r0 <= raw[i][0] < r0 + ps]
            idxs.sort(key=lambda i: -raw[i][1])
            bins = []
            for i in idxs:
                Mt = raw[i][1]
                for b in bins:
                    if b[0] + Mt <= 128:
                        b[1].append(i)
                        b[0] += Mt
                        break
                else:
                    bins.append([Mt, [i]])
            seq.extend(bins)
            r0 += ps
    # coef offsets in processing order + row offsets within each bin
    cslots = []
    cum = 0
    placed = {}
    for b in seq:
        out = []
        ro = 0
        for i in b[1]:
            g, Mt, s0 = raw[i]
            out.append([g, cum, Mt, ro, False, False, s0])
            placed[i] = (len(cslots), len(out) - 1)
            cum += Mt
            ro += Mt
        cslots.append(out)
    Ltot = cum
    # first/last flags per slot by processing (cslot) order
    by_slot = {}
    for ci, cs in enumerate(cslots):
        for ji, u in enumerate(cs):
            by_slot.setdefault(u[0], []).append((ci, ji))
    for g, lst in by_slot.items():
        lst.sort()
        ci, ji = lst[0]
        cslots[ci][ji][4] = True
        ci, ji = lst[-1]
        cslots[ci][ji][5] = True
    cslots = [[tuple(u) + (False,) for u in cs] for cs in cslots]
    # slot completion order -> psO groups
    comp = {}
    for ci, cs in enumerate(cslots):
        for u in cs:
            comp[u[0]] = max(comp.get(u[0], -1), ci)
    order = sorted(range(len(L_slots)), key=lambda g: (comp[g], g))
    grp_of = [0] * len(L_slots)
    gg_of = [0] * len(L_slots)
    gsz_of = [0] * len(L_slots)
    gstart = [0]
    for sz in gsizes:
        gstart.append(gstart[-1] + sz)
    for pos, g in enumerate(order):
        q = 0
        while gstart[q + 1] <= pos:
            q += 1
        grp_of[g] = q
        gg_of[g] = pos - gstart[q]
        gsz_of[g] = gsizes[q]
    return cslots, grp_of, gg_of, gsz_of, gsizes, Ltot


def _build_nc(L_slots):
    import concourse.bacc as bacc
    import concourse.tile as tile
    from concourse import mybir

    L_slots = [int(x) for x in L_slots]
    VN = len(L_slots)
    cslots, grp_of, gg_of, gsz_of, gsizes, Ltot = _pack_units(L_slots)
    NG = len(gsizes)
    units = [u for cs in cslots for u in cs]
    NU = len(units)
    NCS = len(cslots)
    # first tile is a single bank (PERBANK cslots) so the first activation
    # fires as soon as possible after the first COEF chunk lands
    tb = [0, min(PERBANK, NCS)]
    while tb[-1] < NCS:
        tb.append(min(tb[-1] + UPT, NCS))
    tiles_u = [(tb[i], cslots[tb[i]:tb[i + 1]]) for i in range(len(tb) - 1)]

    nc = bacc.Bacc("TRN2", target_bir_lowering=False, debug=False,
                   num_devices=NCORES)
    f32 = mybir.dt.float32
    bf16 = mybir.dt.bfloat16
    f16 = mybir.dt.float16
    COEF = nc.dram_tensor("COEF", [KF, GPTS + Ltot], bf16,
                          kind="ExternalInput")
    SEMP = nc.dram_tensor("SEMP", [128, NU * C], f16, kind="ExternalInput")
    OUT = nc.dram_tensor("OUT", [OROWS, NG * OCOLS], f16,
                         kind="ExternalOutput")

    usem = {}
    for ui_, u_ in enumerate(units):
        usem[(u_[0], u_[1])] = ui_

    # drain each psO group right after its last-emitted unit
    last_pos = {}
    for ci_, cs_ in enumerate(cslots):
        for ji_, u_ in enumerate(cs_):
            last_pos[grp_of[u_[0]]] = (ci_, ji_)
    drain_at = {}
    for grp_, pos_ in last_pos.items():
        drain_at.setdefault(pos_, []).append(grp_)

    # coef-chunk boundaries at cslot granularity (processing order):
    # chunk 0 = first tile (HWDGE, lands first), chunk 1 = next ~2 tiles
    # (SWDGE on the idle Pool engine, generated in parallel), chunks 2/3 =
    # the rest split in two (HWDGE) so no tile waits on one huge transfer.
    def _coff(ci):
        return cslots[ci][0][1] if ci < NCS else Ltot
    c1 = min(PERBANK, NCS)
    c2 = min(c1 + 2 * UPT, NCS)
    c3 = min(c2 + 4 * UPT, NCS)
    cb = [0, _coff(c1), _coff(c2), _coff(c3), Ltot]
    # semp split: first half covers units used by the first ~5 tiles
    cs_lens = [len(cs) for cs in cslots]
    u_s0 = sum(cs_lens[:min(5 * UPT, NCS)])
    u_s0 = max(1, min(NU, u_s0))

    with tile.TileContext(nc) as tc:
        with (
            tc.tile_pool(name="big", bufs=1) as big_pool,
            tc.tile_pool(name="w", bufs=6) as w_pool,
            tc.tile_pool(name="og", bufs=4) as og_pool,
            tc.tile_pool(name="psE", bufs=3, space="PSUM") as pse_pool,
            tc.tile_pool(name="psO", bufs=2, space="PSUM") as pso_pool,
        ):
            coefx_b = big_pool.tile([KF, GPTS + Ltot], bf16)
            rhs_b = coefx_b[:, 0:GPTS]
            coef_b = coefx_b[:, GPTS:]
            semp_b = big_pool.tile([128, NU * C], f16)
            scr_b = big_pool.tile([1, 512], bf16)
            scr_o = big_pool.tile([1, 8], f16)

            # --- warmup: preload the Exp activation table while inputs load
            nc.tensor.write(scr_b[0:1, 0:1],
                            np.zeros(1, ml_dtypes.bfloat16).tobytes())
            nc.scalar.activation(scr_o[0:1, 0:1], scr_b[0:1, 0:1],
                                 mybir.ActivationFunctionType.Exp)

            # --- input loads: chunk0 on SP HWDGE (first to land), tiles 1-2
            # + late semp on Pool SWDGE (parallel generation), rest on SP.
            nc.sync.dma_start(coefx_b[:, 0:GPTS + cb[1]],
                              COEF[:, 0:GPTS + cb[1]])
            if cb[1] < cb[2]:
                nc.gpsimd.dma_start(coef_b[:, cb[1]:cb[2]],
                                    COEF[:, GPTS + cb[1]:GPTS + cb[2]])
            nc.sync.dma_start(semp_b[:, 0:u_s0 * C], SEMP[:, 0:u_s0 * C])
            if u_s0 < NU:
                nc.gpsimd.dma_start(semp_b[:, u_s0 * C:], SEMP[:, u_s0 * C:])
            if cb[2] < cb[3]:
                nc.sync.dma_start(coef_b[:, cb[2]:cb[3]],
                                  COEF[:, GPTS + cb[2]:GPTS + cb[3]])
            if cb[3] < Ltot:
                nc.sync.dma_start(coef_b[:, cb[3]:cb[4]],
                                  COEF[:, GPTS + cb[3]:GPTS + cb[4]])

            pso_t = {}    # group -> psO tile
            w_ts, psEs = {}, {}

            def emit_E(t):
                toff, tu = tiles_u[t]
                psE = pse_pool.tile([128, NBANK, 512], f32, name=f"psE{t}",
                                    tag="psE")
                psEs[t] = psE
                for j, cs in enumerate(tu):
                    b, s = j // PERBANK, j % PERBANK
                    off0 = cs[0][1]
                    rows = sum(u[2] for u in cs)
                    nc.tensor.matmul(
                        psE[0:rows, b:b + 1, s * GPTS:(s + 1) * GPTS],
                        coef_b[:, off0:off0 + rows], rhs_b[:],
                        start=True, stop=True, skip_group_check=True)

            def emit_act(t):
                toff, tu = tiles_u[t]
                psE = psEs[t]
                w_t = w_pool.tile([128, NBANK, OCOLS], f16, name=f"w{t}",
                                  tag="w")
                w_ts[t] = w_t
                mtmax = max(sum(u[2] for u in cs) for cs in tu)
                nb_full, rem = divmod(len(tu), PERBANK)
                if nb_full:
                    nc.scalar.activation(
                        w_t[0:mtmax, 0:nb_full, :],
                        psE[0:mtmax, 0:nb_full, 0:OCOLS],
                        mybir.ActivationFunctionType.Exp)
                if rem:
                    nc.scalar.activation(
                        w_t[0:mtmax, nb_full:nb_full + 1, 0:rem * GPTS],
                        psE[0:mtmax, nb_full:nb_full + 1, 0:rem * GPTS],
                        mybir.ActivationFunctionType.Exp)

            def emit_out(t):
                toff, tu = tiles_u[t]
                w_t = w_ts[t]
                for j, cs in enumerate(tu):
                  b, js = j // PERBANK, j % PERBANK
                  rows = sum(u[2] for u in cs)
                  for ji, (g, off, Mt, po, first, last, s0, pd) \
                          in enumerate(cs):
                    grp, gg, gsz = grp_of[g], gg_of[g], gsz_of[g]
                    gi, s = gg // PERBANK, gg % PERBANK
                    ui = usem[(g, off)]
                    if grp not in pso_t:
                        pso_t[grp] = pso_pool.tile([OROWS, OCOLS], f32,
                                                   name=f"psO{grp}",
                                                   tag="psO")
                    nc.tensor.matmul(
                        pso_t[grp][gi * 32:gi * 32 + C,
                                   s * GPTS:(s + 1) * GPTS],
                        semp_b[0:rows, ui * C:(ui + 1) * C],
                        w_t[0:rows, b:b + 1,
                            js * GPTS:(js + 1) * GPTS],
                        start=first, stop=last, skip_group_check=True)
                    for grp_d in drain_at.get((toff + j, ji), []):
                        gsz_d = gsizes[grp_d]
                        top = (gsz_d - 1) // PERBANK   # last stripe index
                        grows = top * 32 + C
                        gcols = OCOLS if top > 0 else gsz_d * GPTS
                        outg = og_pool.tile([OROWS, OCOLS], f16,
                                            name=f"og{grp_d}", tag="og")
                        # keep the final drain's DMA alone on HWDGE; all
                        # earlier ones generate on the idle Pool engine
                        dma_eng = (nc.sync if grp_d == NG - 1
                                   else nc.gpsimd)
                        nc.vector.tensor_copy(
                            outg[0:grows, 0:gcols],
                            pso_t[grp_d][0:grows, 0:gcols])
                        dma_eng.dma_start(
                            OUT[0:grows,
                                grp_d * OCOLS:grp_d * OCOLS + gcols],
                            outg[0:grows, 0:gcols])

            emit_E(0)
            for t in range(len(tiles_u)):
                emit_act(t)
                if t + 1 < len(tiles_u):
                    emit_E(t + 1)
                emit_out(t)
    nc.compile()
    return nc


def _get_nc(L_slots):
    key = tuple(int(x) for x in L_slots)
    if key not in _NC_CACHE:
        _NC_CACHE[key] = _build_nc(L_slots)
    return _NC_CACHE[key]


def _get_runner(nc):
    """Cached shard_map-jitted executor for one Bass program (axon/PJRT path).

    Mirrors concourse.bass2jax.run_bass_via_pjrt but keeps the jitted callable
    so repeated runs don't rebuild/recompile."""
    if id(nc) in _JIT_CACHE:
        return _JIT_CACHE[id(nc)]
    import jax
    from concourse import bass2jax, mybir
    from jax.experimental.shard_map import shard_map
    from jax.sharding import Mesh, PartitionSpec

    bass2jax.install_neuronx_cc_hook()
    partition_name = (nc.partition_id_tensor.name
                      if nc.partition_id_tensor else None)
    in_names, out_names, out_avals, zero_outs = [], [], [], []
    for alloc in nc.m.functions[0].allocations:
        if not isinstance(alloc, mybir.MemoryLocationSet):
            continue
        name = alloc.memorylocations[0].name
        if alloc.kind == "ExternalInput":
            if name == partition_name:
                continue
            in_names.append(name)
        elif alloc.kind == "ExternalOutput":
            shape = tuple(alloc.tensor_shape)
            dtype = mybir.dt.np(alloc.dtype)
            out_names.append(name)
            out_avals.append(jax.core.ShapedArray(shape, dtype))
            zero_outs.append(np.zeros(shape, dtype))
    n_params = len(in_names)
    all_in_names = in_names + out_names
    if partition_name is not None:
        all_in_names = all_in_names + [partition_name]

    def _body(*args):
        operands = list(args)
        if partition_name is not None:
            operands.append(bass2jax.partition_id_tensor())
        outs = bass2jax._bass_exec_p.bind(
            *operands,
            out_avals=tuple(out_avals),
            in_names=tuple(all_in_names),
            out_names=tuple(out_names),
            lowering_input_output_aliases=(),
            sim_require_finite=True,
            sim_require_nnan=True,
            nc=nc,
        )
        return tuple(outs)

    devices = jax.devices()[:NCORES]
    mesh = Mesh(np.asarray(devices), ("core",))
    donate = tuple(range(n_params, n_params + len(out_names)))
    sharded = jax.jit(
        shard_map(_body, mesh=mesh,
                  in_specs=(PartitionSpec("core"),) * (n_params + len(out_names)),
                  out_specs=(PartitionSpec("core"),) * len(out_names),
                  check_rep=False),
        donate_argnums=donate, keep_unused=True)

    def run(in_maps, rounds=1):
        concat_in = [np.concatenate([np.asarray(m[nm]) for m in in_maps], axis=0)
                     for nm in in_names]
        outs = None
        for _ in range(rounds):
            zo = [np.concatenate([z] * NCORES, axis=0) for z in zero_outs]
            outs = sharded(*concat_in, *zo)
        outs = [np.asarray(o) for o in outs]
        results = []
        for ci in range(NCORES):
            d = {}
            for oi, nm in enumerate(out_names):
                per = outs[oi].shape[0] // NCORES
                d[nm] = outs[oi][ci * per:(ci + 1) * per]
            results.append(d)
        return results, sharded, (concat_in, zero_outs, in_names, out_names)

    sharded_nd = jax.jit(
        shard_map(_body, mesh=mesh,
                  in_specs=(PartitionSpec("core"),) * (n_params + len(out_names)),
                  out_specs=(PartitionSpec("core"),) * len(out_names),
                  check_rep=False),
        keep_unused=True)

    def timeit(in_maps, iters=30):
        import time as _time
        from jax.sharding import NamedSharding
        sh = NamedSharding(mesh, PartitionSpec("core"))
        concat_in = [np.concatenate([np.asarray(m[nm]) for m in in_maps], axis=0)
                     for nm in in_names]
        zo = [np.concatenate([z] * NCORES, axis=0) for z in zero_outs]
        args = [jax.device_put(a, sh) for a in concat_in + zo]
        outs = sharded_nd(*args)
        jax.block_until_ready(outs)
        t0 = _time.time()
        for _ in range(iters):
            outs = sharded_nd(*args)
        jax.block_until_ready(outs)
        return (_time.time() - t0) / iters

    run.timeit = timeit
    _JIT_CACHE[id(nc)] = run
    return run


def _bf16_split(x):
    hi = x.astype(ml_dtypes.bfloat16).astype(np.float32)
    return hi, (x - hi).astype(np.float32)


def _host_prep(pts, means3D, opacities, semantics, scales, cov3D, origin_use):
    pts = np.asarray(pts, np.float32).reshape(N, 3)
    mu32 = np.asarray(means3D, np.float32).reshape(P, 3)
    op = np.asarray(opacities, np.float64).reshape(P)
    sem = np.asarray(semantics, np.float32).reshape(P, C)
    sc32 = np.asarray(scales, np.float32).reshape(P, 3)
    cov = np.asarray(cov3D, np.float64).reshape(P, 3, 3)
    org32 = np.asarray(origin_use, np.float32).reshape(3)

    # --- integer binning, replicated in fp32 exactly like the reference ---
    radii = np.ceil(sc32.max(-1) * np.float32(SCALE_MULT) / np.float32(GRID)
                    ).astype(np.int32).astype(np.int64)
    m_int = ((mu32 - org32) / np.float32(GRID)).astype(np.int32).astype(np.int64)
    p_int = ((pts - org32) / np.float32(GRID)).astype(np.int32).astype(np.int64)

    # structured-input check: points must be the (i, j, k) voxel-center grid
    idx = np.arange(N)
    kk = idx % D
    col = idx // D
    jj = col % W
    ii = col // W
    grid_int = np.stack([ii, jj, kk], axis=-1)
    if not np.array_equal(p_int, grid_int):
        raise RuntimeError("kernel: unstructured pts not supported by fast path")

    # --- per-Gaussian inverse covariance (float64) ---
    a, b, c_, d, e, f = (cov[:, 0, 0], cov[:, 1, 1], cov[:, 2, 2],
                         cov[:, 0, 1], cov[:, 1, 2], cov[:, 0, 2])
    det = a * (b * c_ - e * e) - d * (d * c_ - e * f) + f * (d * e - b * f)
    ixx = (b * c_ - e * e) / det
    iyy = (a * c_ - f * f) / det
    izz = (a * b - d * d) / det
    ixy = (e * f - d * c_) / det
    iyz = (d * f - a * e) / det
    ixz = (d * e - b * f) / det
    A = np.empty((P, 3, 3))
    A[:, 0, 0], A[:, 1, 1], A[:, 2, 2] = ixx, iyy, izz
    A[:, 0, 1] = A[:, 1, 0] = ixy
    A[:, 1, 2] = A[:, 2, 1] = iyz
    A[:, 0, 2] = A[:, 2, 0] = ixz
    mu = mu32.astype(np.float64)
    logop = np.log(op)

    # --- blocks: bounds, hit lists, load-balanced assignment ---
    mx, my, mz = m_int[:, 0], m_int[:, 1], m_int[:, 2]
    blocks = []            # (x0, y0, z0)
    hits = []
    for bxi in range(NBX):
        for byi in range(NBY):
            for bzi in range(NBZ):
                x0, y0, z0 = bxi * BX, byi * BY, bzi * BZ
                hit = np.where(
                    (mx + radii >= x0) & (mx - radii <= x0 + BX - 1) &
                    (my + radii >= y0) & (my - radii <= y0 + BY - 1) &
                    (mz + radii >= z0) & (mz - radii <= z0 + BZ - 1))[0]
                blocks.append((x0, y0, z0))
                hits.append(hit)
    Ls = np.array([len(h) for h in hits])
    order = np.argsort(-Ls, kind="stable")     # rank r -> block id
    # slot k <- rank seq_ranks[k]: deal ranks from thirds so every
    # region of slots mixes large/mid/small hit lists
    t3 = (NSLOT + 2) // 3
    seq_ranks = []
    for i in range(t3):
        for base in (0, t3, 2 * t3):
            if base + i < NSLOT:
                seq_ranks.append(base + i)
    # core c, slot k gets block order[8*seq_ranks[k] + c].  Slots with
    # more than 128 hits are split into independent virtual slots so
    # every psO accumulation is a single start+stop matmul.
    raw_L = [max(1, int(Ls[order[8 * seq_ranks[k]]])) for k in range(NSLOT)]
    L_slots, vmap = [], []
    for k, L in enumerate(raw_L):
        L_slots.append(min(128, L))
        vmap.append((k, 0))
        if L > 128:
            L_slots.append(L - 128)
            vmap.append((k, 128))
    VN = len(L_slots)
    cslots, grp_of, gg_of, gsz_of, gsizes, Ltot = _pack_units(L_slots)
    units = [u for cs in cslots for u in cs]
    NU = len(units)

    # --- shared RHS: block-local features (hi/lo pairs) + one-hot rows ---
    lx = np.arange(GPTS) // (BY * BZ)
    ly = (np.arange(GPTS) // BZ) % BY
    lz = np.arange(GPTS) % BZ
    xi = ((lx - (BX - 1) / 2.0) * GRID).astype(np.float32)
    yi = ((ly - (BY - 1) / 2.0) * GRID).astype(np.float32)
    zi = ((lz - (BZ - 1) / 2.0) * GRID).astype(np.float32)
    feat10 = np.stack([xi * xi, yi * yi, zi * zi, xi * yi, yi * zi, xi * zi,
                       xi, yi, zi, np.ones(GPTS, np.float32)])
    fhi, flo = _bf16_split(feat10)
    rhs = np.zeros((KF, GPTS), np.float32)
    rhs[0:10] = fhi
    rhs[10:20] = flo
    rhs[20:30] = fhi
    rhs[30 + lx, np.arange(GPTS)] = 1.0
    rhs[30 + BX + ly, np.arange(GPTS)] = 1.0
    rhs[30 + BX + BY + lz, np.arange(GPTS)] = 1.0
    rhs = rhs.astype(ml_dtypes.bfloat16)

    # --- per-core COEF / SEMP ---
    in_maps = []
    perm_blocks = []       # per core: slot -> block id
    # per-slot units (in processing order they appear)
    units_of_slot = {}
    for u in units:
        units_of_slot.setdefault(u[0], []).append(u)
    for ci in range(NCORES):
        coef_m = np.zeros((KF, Ltot), np.float32)
        semp = np.zeros((128, NU * C), np.float16)
        my_blocks = []
        uindex = {}
        for ui, u in enumerate(units):
            uindex[(u[0], u[1])] = ui
        for g in range(VN):
            k, vs0 = vmap[g]
            bid = int(order[8 * seq_ranks[k] + ci])
            my_blocks.append((bid, vs0))
            hit = hits[bid]
            nh = len(hit)
            if nh == 0 or vs0 >= nh:
                continue
            x0, y0, z0 = blocks[bid]
            cx = (x0 + (BX - 1) / 2.0 + 0.5) * GRID
            cy = (y0 + (BY - 1) / 2.0 + 0.5) * GRID
            cz = (z0 + (BZ - 1) / 2.0 + 0.5) * GRID
            dmu = mu[hit] - np.array([cx, cy, cz])      # [nh, 3]
            Ah = A[hit]                                  # [nh, 3, 3]
            Amu = np.einsum('pij,pj->pi', Ah, dmu)
            muAmu = np.einsum('pi,pi->p', dmu, Amu)
            c10 = np.stack([
                -0.5 * Ah[:, 0, 0], -0.5 * Ah[:, 1, 1], -0.5 * Ah[:, 2, 2],
                -Ah[:, 0, 1], -Ah[:, 1, 2], -Ah[:, 0, 2],
                Amu[:, 0], Amu[:, 1], Amu[:, 2],
                -0.5 * muAmu + logop[hit]]).astype(np.float32)
            chi, clo = _bf16_split(c10)
            in_x = (np.abs(x0 + np.arange(BX)[None, :] - mx[hit, None])
                    <= radii[hit, None])
            in_y = (np.abs(y0 + np.arange(BY)[None, :] - my[hit, None])
                    <= radii[hit, None])
            in_z = (np.abs(z0 + np.arange(BZ)[None, :] - mz[hit, None])
                    <= radii[hit, None])
            px = np.where(in_x, 0.0, PEN).T
            py = np.where(in_y, 0.0, PEN).T
            pz = np.where(in_z, 0.0, PEN).T
            semh = sem[hit].astype(np.float16)
            for (gg_, off, Mt, po, first, last, s0, pd) in units_of_slot[g]:
                a, b2 = vs0 + s0, min(vs0 + s0 + Mt, nh)
                if a >= nh:
                    continue
                w_ = b2 - a
                coef_m[0:10, off:off + w_] = chi[:, a:b2]
                coef_m[10:20, off:off + w_] = chi[:, a:b2]
                coef_m[20:30, off:off + w_] = clo[:, a:b2]
                coef_m[30:30 + BX, off:off + w_] = px[:, a:b2]
                coef_m[30 + BX:30 + BX + BY, off:off + w_] = py[:, a:b2]
                coef_m[30 + BX + BY:, off:off + w_] = pz[:, a:b2]
                ui = uindex[(gg_, off)]
                semp[po:po + w_, ui * C:(ui + 1) * C] = semh[a:b2]
        perm_blocks.append(my_blocks)
        coefx = np.concatenate([np.asarray(rhs, np.float32),
                                coef_m], axis=1).astype(ml_dtypes.bfloat16)
        in_maps.append({"COEF": coefx, "SEMP": semp})
    return in_maps, L_slots, (perm_blocks, grp_of, gg_of)


def kernel(**inputs):
    in_maps, L_slots, (perm_blocks, grp_of, gg_of) = _host_prep(**inputs)
    nc = _get_nc(L_slots)
    run = _get_runner(nc)
    results, _, _ = run(in_maps)
    out = np.empty((N, C), np.float32)
    lx = np.arange(GPTS) // (BY * BZ)
    ly = (np.arange(GPTS) // BZ) % BY
    lz = np.arange(GPTS) % BZ
    for ci in range(NCORES):
        o = results[ci]["OUT"].astype(np.float32)
        for g in range(len(perm_blocks[ci])):
            bid, vs0 = perm_blocks[ci][g]
            x0 = (bid // (NBY * NBZ)) * BX
            y0 = ((bid // NBZ) % NBY) * BY
            z0 = (bid % NBZ) * BZ
            grp, gg = grp_of[g], gg_of[g]
            gi, s = gg // PERBANK, gg % PERBANK
            col0 = grp * OCOLS + s * GPTS
            blk = o[gi * 32:gi * 32 + C, col0:col0 + GPTS]   # [C, GPTS]
            gidx = ((x0 + lx) * W + (y0 + ly)) * D + (z0 + lz)
            if vs0 == 0:
                out[gidx] = blk.T
            else:
                out[gidx] += blk.T
    return out



# revision 19
# speedup vs baseline: 1.0325x; 1.0154x over previous
# Trainium2 Bass kernel for nn_LocalAggregator (Gaussian -> voxel-grid semantic
# compositing).
#
# Strategy: the voxel grid (60,60,36) is tiled into 1080 3-D blocks of
# (4,5,6) = 120 voxels.  Blocks are dealt to the 8 cores by sorted hit-count
# (rank 8k+c -> core c, slot k) so every core sees a near-identical load
# profile; the host un-permutes the output afterwards.  For each block the
# host builds the exact list of Gaussians whose integer box overlaps the
# block in all three dims.  In block-local coordinates the feature matrix
# (quadratic monomials + one-hot rows for the x/y/z box tests) is the SAME
# for every block, so a single small RHS is shared by all matmuls; all
# per-(block,Gaussian) data lives in the COEF matrix.  The E matmul runs in
# bf16 with a 3-way hi/lo product split (rows [hi,hi,lo] x features
# [fhi,flo,fhi] drop only the lo*lo term, ~1e-5 relative), Exp runs on the
# scalar engine over 12 units (3 PSUM banks x 4 blocks each) at a time, and
# the semantic reduction is an fp16 matmul accumulated into PSUM banks packed
# 12 blocks deep (3 partition stripes x 4 column slots) so one DVE copy +
# one Pool-engine DMA drains 12 blocks at once.  Dummy matmuls at t=0 ramp
# the PE clock to max p-state and a dummy activation preloads the Exp table.
import numpy as np
import ml_dtypes

H, W, D = 60, 60, 36
GRID = 0.08
SCALE_MULT = 3.0
P = 2048
C = 13
N = H * W * D                  # 129600
NCORES = 8
BX, BY, BZ = 4, 5, 6           # block shape
NBX, NBY, NBZ = H // BX, W // BY, D // BZ
NB = NBX * NBY * NBZ           # 1080 blocks total
GPTS = BX * BY * BZ            # 120 points per block
NSLOT = NB // NCORES           # 135 blocks (slots) per core
NPC = NSLOT * GPTS             # 16200 points per core
KF = 30 + BX + BY + BZ         # 45 feature rows (3x10 split products + onehots)
PEN = -2000.0                  # box-miss penalty (exp() == 0 in fp32)
PERBANK = 4                    # 120-col units per 512-f32 PSUM bank
NBANK = 2                      # banks per psE tile / act instruction
UPT = PERBANK * NBANK          # 8 cslots per psE tile
OGRP = 12                      # blocks per psO bank (3 stripes x 4 col slots)
OROWS = 77                     # 2*32+13 live partitions per output group
OCOLS = PERBANK * GPTS         # 480 cols per output group
NGRP = (NSLOT + OGRP - 1) // OGRP   # output groups per core (last partial)

_NC_CACHE: dict = {}
_JIT_CACHE: dict = {}


def _pack_units(L_slots):
    """Bin-pack units into 128-partition column-slots.

    All E matmuls share one RHS, so any units whose COEF columns are laid
    out adjacently can share a single E matmul (M = sum Mt <= 128) and one
    activation column range.  The per-block semantic matmul then contracts
    K = bin rows from base partition 0; zero semantics rows mask the other
    blocks in the bin.  Returns (cslots, grp_of, gg_of, gsz_of, Ltot) with
    cslots a list of bins, each a list of
    (slot, coef_off, Mt, row_off, first_in_slot, last_in_slot, s0).
    psO groups follow the slot completion order; bins are ordered to keep
    the per-activation-tile unit count flat.
    """
    raw = []
    for g, L in enumerate(L_slots):
        L = int(L)
        for s in range(0, L, 128):
            raw.append((g, min(128, L - s), s))       # (slot, Mt, s0)
    # psO groups in completion order: OGRP-slot groups, plus a tiny final
    # group so the very last drain's copy+DMA chain is short.  Packing
    # regions = consecutive group PAIRS (psO bufs = 2), so a region's bins
    # never reference a psO tile that hasn't been freed by an earlier drain.
    VN_ = len(L_slots)
    k, rem1 = divmod(VN_, OGRP)
    if k == 0:
        gsizes = [rem1]
    elif rem1:
        gsizes = [OGRP] * k + [rem1]
    else:
        gsizes = [OGRP] * (k - 1) + [OGRP - 4, 4]
    # packing regions: pairs of groups, except the final two groups pack
    # SEQUENTIALLY (all of group NG-2's units strictly before group NG-1's)
    # so only the small final group's drain sits on the tail critical path
    regions = []          # (n_slots, sequential_subphases)
    i = 0
    while i < len(gsizes):
        if i + 2 >= len(gsizes):
            regions.append((sum(gsizes[i:]), gsizes[i:]))
            break
        regions.append((gsizes[i] + gsizes[i + 1], None))
        i += 2
    seq = []
    r0 = 0
    for rs, phases in regions:
        for ps in (phases or [rs]):
            idxs = [i for i in range(len(raw))
                    if r0 <= raw[i][0] < r0 + ps]
            idxs.sort(key=lambda i: -raw[i][1])
            bins = []
            for i in idxs:
                Mt = raw[i][1]
                for b in bins:
                    if b[0] + Mt <= 128:
                        b[1].append(i)
                        b[0] += Mt
                        break
                else:
                    bins.append([Mt, [i]])
            seq.extend(bins)
            r0 += ps
    # coef offsets in processing order + row offsets within each bin
    cslots = []
    cum = 0
    placed = {}
    for b in seq:
        out = []
        ro = 0
        for i in b[1]:
            g, Mt, s0 = raw[i]
            out.append([g, cum, Mt, ro, False, False, s0])
            placed[i] = (len(cslots), len(out) - 1)
            cum += Mt
            ro += Mt
        cslots.append(out)
    Ltot = cum
    # first/last flags per slot by processing (cslot) order
    by_slot = {}
    for ci, cs in enumerate(cslots):
        for ji, u in enumerate(cs):
            by_slot.setdefault(u[0], []).append((ci, ji))
    for g, lst in by_slot.items():
        lst.sort()
        ci, ji = lst[0]
        cslots[ci][ji][4] = True
        ci, ji = lst[-1]
        cslots[ci][ji][5] = True
    cslots = [[tuple(u) + (False,) for u in cs] for cs in cslots]
    # slot completion order -> psO groups
    comp = {}
    for ci, cs in enumerate(cslots):
        for u in cs:
            comp[u[0]] = max(comp.get(u[0], -1), ci)
    order = sorted(range(len(L_slots)), key=lambda g: (comp[g], g))
    grp_of = [0] * len(L_slots)
    gg_of = [0] * len(L_slots)
    gsz_of = [0] * len(L_slots)
    gstart = [0]
    for sz in gsizes:
        gstart.append(gstart[-1] + sz)
    for pos, g in enumerate(order):
        q = 0
        while gstart[q + 1] <= pos:
            q += 1
        grp_of[g] = q
        gg_of[g] = pos - gstart[q]
        gsz_of[g] = gsizes[q]
    return cslots, grp_of, gg_of, gsz_of, gsizes, Ltot


def _build_nc(L_slots):
    import concourse.bacc as bacc
    import concourse.tile as tile
    from concourse import mybir

    L_slots = [int(x) for x in L_slots]
    VN = len(L_slots)
    cslots, grp_of, gg_of, gsz_of, gsizes, Ltot = _pack_units(L_slots)
    NG = len(gsizes)
    units = [u for cs in cslots for u in cs]
    NU = len(units)
    NCS = len(cslots)
    # first tile is a single bank (PERBANK cslots) so the first activation
    # fires as soon as possible after the first COEF chunk lands
    tb = [0, min(PERBANK, NCS)]
    while tb[-1] < NCS:
        tb.append(min(tb[-1] + UPT, NCS))
    tiles_u = [(tb[i], cslots[tb[i]:tb[i + 1]]) for i in range(len(tb) - 1)]

    nc = bacc.Bacc("TRN2", target_bir_lowering=False, debug=False,
                   num_devices=NCORES)
    f32 = mybir.dt.float32
    bf16 = mybir.dt.bfloat16
    f16 = mybir.dt.float16
    COEF = nc.dram_tensor("COEF", [KF, GPTS + Ltot], bf16,
                          kind="ExternalInput")
    SEMP = nc.dram_tensor("SEMP", [128, NU * C], f16, kind="ExternalInput")
    OUT = nc.dram_tensor("OUT", [OROWS, NG * OCOLS], f16,
                         kind="ExternalOutput")

    usem = {}
    for ui_, u_ in enumerate(units):
        usem[(u_[0], u_[1])] = ui_

    # drain each psO group right after its last-emitted unit
    last_pos = {}
    for ci_, cs_ in enumerate(cslots):
        for ji_, u_ in enumerate(cs_):
            last_pos[grp_of[u_[0]]] = (ci_, ji_)
    drain_at = {}
    for grp_, pos_ in last_pos.items():
        drain_at.setdefault(pos_, []).append(grp_)

    # coef-chunk boundaries at cslot granularity (processing order):
    # chunk 0 = first tile (HWDGE, lands first), chunk 1 = next ~2 tiles
    # (SWDGE on the idle Pool engine, generated in parallel), chunks 2/3 =
    # the rest split in two (HWDGE) so no tile waits on one huge transfer.
    def _coff(ci):
        return cslots[ci][0][1] if ci < NCS else Ltot
    c1 = min(PERBANK, NCS)
    c2 = min(c1 + 2 * UPT, NCS)
    c3 = min(c2 + 4 * UPT, NCS)
    cb = [0, _coff(c1), _coff(c2), _coff(c3), Ltot]
    # semp split: first half covers units used by the first ~5 tiles
    cs_lens = [len(cs) for cs in cslots]
    u_s0 = sum(cs_lens[:min(5 * UPT, NCS)])
    u_s0 = max(1, min(NU, u_s0))

    with tile.TileContext(nc) as tc:
        with (
            tc.tile_pool(name="big", bufs=1) as big_pool,
            tc.tile_pool(name="w", bufs=6) as w_pool,
            tc.tile_pool(name="og", bufs=6) as og_pool,
            tc.tile_pool(name="psE", bufs=3, space="PSUM") as pse_pool,
            tc.tile_pool(name="psO", bufs=2, space="PSUM") as pso_pool,
        ):
            coefx_b = big_pool.tile([KF, GPTS + Ltot], bf16)
            rhs_b = coefx_b[:, 0:GPTS]
            coef_b = coefx_b[:, GPTS:]
            semp_b = big_pool.tile([128, NU * C], f16)
            scr_b = big_pool.tile([1, 512], bf16)
            scr_o = big_pool.tile([1, 8], f16)

            # --- warmup: preload the Exp activation table while inputs load
            nc.tensor.write(scr_b[0:1, 0:1],
                            np.zeros(1, ml_dtypes.bfloat16).tobytes())
            nc.scalar.activation(scr_o[0:1, 0:1], scr_b[0:1, 0:1],
                                 mybir.ActivationFunctionType.Exp)

            # --- input loads: chunk0 on SP HWDGE (first to land), tiles 1-2
            # + late semp on Pool SWDGE (parallel generation), rest on SP.
            nc.sync.dma_start(coefx_b[:, 0:GPTS + cb[1]],
                              COEF[:, 0:GPTS + cb[1]])
            if cb[1] < cb[2]:
                nc.gpsimd.dma_start(coef_b[:, cb[1]:cb[2]],
                                    COEF[:, GPTS + cb[1]:GPTS + cb[2]])
            nc.sync.dma_start(semp_b[:, 0:u_s0 * C], SEMP[:, 0:u_s0 * C])
            if u_s0 < NU:
                nc.gpsimd.dma_start(semp_b[:, u_s0 * C:], SEMP[:, u_s0 * C:])
            if cb[2] < cb[3]:
                nc.sync.dma_start(coef_b[:, cb[2]:cb[3]],
                                  COEF[:, GPTS + cb[2]:GPTS + cb[3]])
            if cb[3] < Ltot:
                nc.sync.dma_start(coef_b[:, cb[3]:cb[4]],
                                  COEF[:, GPTS + cb[3]:GPTS + cb[4]])

            pso_t = {}    # group -> psO tile
            w_ts, psEs = {}, {}

            def emit_E(t):
                toff, tu = tiles_u[t]
                psE = pse_pool.tile([128, NBANK, 512], f32, name=f"psE{t}",
                                    tag="psE")
                psEs[t] = psE
                for j, cs in enumerate(tu):
                    b, s = j // PERBANK, j % PERBANK
                    off0 = cs[0][1]
                    rows = sum(u[2] for u in cs)
                    nc.tensor.matmul(
                        psE[0:rows, b:b + 1, s * GPTS:(s + 1) * GPTS],
                        coef_b[:, off0:off0 + rows], rhs_b[:],
                        start=True, stop=True, skip_group_check=True)

            def emit_act(t):
                toff, tu = tiles_u[t]
                psE = psEs[t]
                w_t = w_pool.tile([128, NBANK, OCOLS], f16, name=f"w{t}",
                                  tag="w")
                w_ts[t] = w_t
                mtmax = max(sum(u[2] for u in cs) for cs in tu)
                nb_full, rem = divmod(len(tu), PERBANK)
                if nb_full:
                    nc.scalar.activation(
                        w_t[0:mtmax, 0:nb_full, :],
                        psE[0:mtmax, 0:nb_full, 0:OCOLS],
                        mybir.ActivationFunctionType.Exp)
                if rem:
                    nc.scalar.activation(
                        w_t[0:mtmax, nb_full:nb_full + 1, 0:rem * GPTS],
                        psE[0:mtmax, nb_full:nb_full + 1, 0:rem * GPTS],
                        mybir.ActivationFunctionType.Exp)

            def emit_out(t):
                toff, tu = tiles_u[t]
                w_t = w_ts[t]
                for j, cs in enumerate(tu):
                  b, js = j // PERBANK, j % PERBANK
                  rows = sum(u[2] for u in cs)
                  for ji, (g, off, Mt, po, first, last, s0, pd) \
                          in enumerate(cs):
                    grp, gg, gsz = grp_of[g], gg_of[g], gsz_of[g]
                    gi, s = gg // PERBANK, gg % PERBANK
                    ui = usem[(g, off)]
                    if grp not in pso_t:
                        pso_t[grp] = pso_pool.tile([OROWS, OCOLS], f32,
                                                   name=f"psO{grp}",
                                                   tag="psO")
                    nc.tensor.matmul(
                        pso_t[grp][gi * 32:gi * 32 + C,
                                   s * GPTS:(s + 1) * GPTS],
                        semp_b[0:rows, ui * C:(ui + 1) * C],
                        w_t[0:rows, b:b + 1,
                            js * GPTS:(js + 1) * GPTS],
                        start=first, stop=last, skip_group_check=True)
                    for grp_d in drain_at.get((toff + j, ji), []):
                        gsz_d = gsizes[grp_d]
                        top = (gsz_d - 1) // PERBANK   # last stripe index
                        grows = top * 32 + C
                        gcols = OCOLS if top > 0 else gsz_d * GPTS
                        outg = og_pool.tile([OROWS, OCOLS], f16,
                                            name=f"og{grp_d}", tag="og")
                        # final drain on HWDGE; earlier ones alternate
                        # Pool-SWDGE / SP-HWDGE so generation parallelizes
                        dma_eng = (nc.sync if (grp_d == NG - 1
                                               or grp_d % 2 == 1)
                                   else nc.gpsimd)
                        nc.vector.tensor_copy(
                            outg[0:grows, 0:gcols],
                            pso_t[grp_d][0:grows, 0:gcols])
                        dma_eng.dma_start(
                            OUT[0:grows,
                                grp_d * OCOLS:grp_d * OCOLS + gcols],
                            outg[0:grows, 0:gcols])

            emit_E(0)
            for t in range(len(tiles_u)):
                emit_act(t)
                if t + 1 < len(tiles_u):
                    emit_E(t + 1)
                emit_out(t)
    nc.compile()
    return nc


def _get_nc(L_slots):
    key = tuple(int(x) for x in L_slots)
    if key not in _NC_CACHE:
        _NC_CACHE[key] = _build_nc(L_slots)
    return _NC_CACHE[key]


def _get_runner(nc):
    """Cached shard_map-jitted executor for one Bass program (axon/PJRT path).

    Mirrors concourse.bass2jax.run_bass_via_pjrt but keeps the jitted callable
    so repeated runs don't rebuild/recompile."""
    if id(nc) in _JIT_CACHE:
        return _JIT_CACHE[id(nc)]
    import jax
    from concourse import bass2jax, mybir
    from jax.experimental.shard_map import shard_map
    from jax.sharding import Mesh, PartitionSpec

    bass2jax.install_neuronx_cc_hook()
    partition_name = (nc.partition_id_tensor.name
                      if nc.partition_id_tensor else None)
    in_names, out_names, out_avals, zero_outs = [], [], [], []
    for alloc in nc.m.functions[0].allocations:
        if not isinstance(alloc, mybir.MemoryLocationSet):
            continue
        name = alloc.memorylocations[0].name
        if alloc.kind == "ExternalInput":
            if name == partition_name:
                continue
            in_names.append(name)
        elif alloc.kind == "ExternalOutput":
            shape = tuple(alloc.tensor_shape)
            dtype = mybir.dt.np(alloc.dtype)
            out_names.append(name)
            out_avals.append(jax.core.ShapedArray(shape, dtype))
            zero_outs.append(np.zeros(shape, dtype))
    n_params = len(in_names)
    all_in_names = in_names + out_names
    if partition_name is not None:
        all_in_names = all_in_names + [partition_name]

    def _body(*args):
        operands = list(args)
        if partition_name is not None:
            operands.append(bass2jax.partition_id_tensor())
        outs = bass2jax._bass_exec_p.bind(
            *operands,
            out_avals=tuple(out_avals),
            in_names=tuple(all_in_names),
            out_names=tuple(out_names),
            lowering_input_output_aliases=(),
            sim_require_finite=True,
            sim_require_nnan=True,
            nc=nc,
        )
        return tuple(outs)

    devices = jax.devices()[:NCORES]
    mesh = Mesh(np.asarray(devices), ("core",))
    donate = tuple(range(n_params, n_params + len(out_names)))
    sharded = jax.jit(
        shard_map(_body, mesh=mesh,
                  in_specs=(PartitionSpec("core"),) * (n_params + len(out_names)),
                  out_specs=(PartitionSpec("core"),) * len(out_names),
                  check_rep=False),
        donate_argnums=donate, keep_unused=True)

    def run(in_maps, rounds=1):
        concat_in = [np.concatenate([np.asarray(m[nm]) for m in in_maps], axis=0)
                     for nm in in_names]
        outs = None
        for _ in range(rounds):
            zo = [np.concatenate([z] * NCORES, axis=0) for z in zero_outs]
            outs = sharded(*concat_in, *zo)
        outs = [np.asarray(o) for o in outs]
        results = []
        for ci in range(NCORES):
            d = {}
            for oi, nm in enumerate(out_names):
                per = outs[oi].shape[0] // NCORES
                d[nm] = outs[oi][ci * per:(ci + 1) * per]
            results.append(d)
        return results, sharded, (concat_in, zero_outs, in_names, out_names)

    sharded_nd = jax.jit(
        shard_map(_body, mesh=mesh,
                  in_specs=(PartitionSpec("core"),) * (n_params + len(out_names)),
                  out_specs=(PartitionSpec("core"),) * len(out_names),
                  check_rep=False),
        keep_unused=True)

    def timeit(in_maps, iters=30):
        import time as _time
        from jax.sharding import NamedSharding
        sh = NamedSharding(mesh, PartitionSpec("core"))
        concat_in = [np.concatenate([np.asarray(m[nm]) for m in in_maps], axis=0)
                     for nm in in_names]
        zo = [np.concatenate([z] * NCORES, axis=0) for z in zero_outs]
        args = [jax.device_put(a, sh) for a in concat_in + zo]
        outs = sharded_nd(*args)
        jax.block_until_ready(outs)
        t0 = _time.time()
        for _ in range(iters):
            outs = sharded_nd(*args)
        jax.block_until_ready(outs)
        return (_time.time() - t0) / iters

    run.timeit = timeit
    _JIT_CACHE[id(nc)] = run
    return run


def _bf16_split(x):
    hi = x.astype(ml_dtypes.bfloat16).astype(np.float32)
    return hi, (x - hi).astype(np.float32)


def _host_prep(pts, means3D, opacities, semantics, scales, cov3D, origin_use):
    pts = np.asarray(pts, np.float32).reshape(N, 3)
    mu32 = np.asarray(means3D, np.float32).reshape(P, 3)
    op = np.asarray(opacities, np.float64).reshape(P)
    sem = np.asarray(semantics, np.float32).reshape(P, C)
    sc32 = np.asarray(scales, np.float32).reshape(P, 3)
    cov = np.asarray(cov3D, np.float64).reshape(P, 3, 3)
    org32 = np.asarray(origin_use, np.float32).reshape(3)

    # --- integer binning, replicated in fp32 exactly like the reference ---
    radii = np.ceil(sc32.max(-1) * np.float32(SCALE_MULT) / np.float32(GRID)
                    ).astype(np.int32).astype(np.int64)
    m_int = ((mu32 - org32) / np.float32(GRID)).astype(np.int32).astype(np.int64)
    p_int = ((pts - org32) / np.float32(GRID)).astype(np.int32).astype(np.int64)

    # structured-input check: points must be the (i, j, k) voxel-center grid
    idx = np.arange(N)
    kk = idx % D
    col = idx // D
    jj = col % W
    ii = col // W
    grid_int = np.stack([ii, jj, kk], axis=-1)
    if not np.array_equal(p_int, grid_int):
        raise RuntimeError("kernel: unstructured pts not supported by fast path")

    # --- per-Gaussian inverse covariance (float64) ---
    a, b, c_, d, e, f = (cov[:, 0, 0], cov[:, 1, 1], cov[:, 2, 2],
                         cov[:, 0, 1], cov[:, 1, 2], cov[:, 0, 2])
    det = a * (b * c_ - e * e) - d * (d * c_ - e * f) + f * (d * e - b * f)
    ixx = (b * c_ - e * e) / det
    iyy = (a * c_ - f * f) / det
    izz = (a * b - d * d) / det
    ixy = (e * f - d * c_) / det
    iyz = (d * f - a * e) / det
    ixz = (d * e - b * f) / det
    A = np.empty((P, 3, 3))
    A[:, 0, 0], A[:, 1, 1], A[:, 2, 2] = ixx, iyy, izz
    A[:, 0, 1] = A[:, 1, 0] = ixy
    A[:, 1, 2] = A[:, 2, 1] = iyz
    A[:, 0, 2] = A[:, 2, 0] = ixz
    mu = mu32.astype(np.float64)
    logop = np.log(op)

    # --- blocks: bounds, hit lists, load-balanced assignment ---
    mx, my, mz = m_int[:, 0], m_int[:, 1], m_int[:, 2]
    blocks = []            # (x0, y0, z0)
    hits = []
    for bxi in range(NBX):
        for byi in range(NBY):
            for bzi in range(NBZ):
                x0, y0, z0 = bxi * BX, byi * BY, bzi * BZ
                hit = np.where(
                    (mx + radii >= x0) & (mx - radii <= x0 + BX - 1) &
                    (my + radii >= y0) & (my - radii <= y0 + BY - 1) &
                    (mz + radii >= z0) & (mz - radii <= z0 + BZ - 1))[0]
                blocks.append((x0, y0, z0))
                hits.append(hit)
    Ls = np.array([len(h) for h in hits])
    order = np.argsort(-Ls, kind="stable")     # rank r -> block id
    # slot k <- rank seq_ranks[k]: deal ranks from thirds so every
    # region of slots mixes large/mid/small hit lists
    t3 = (NSLOT + 2) // 3
    seq_ranks = []
    for i in range(t3):
        for base in (0, t3, 2 * t3):
            if base + i < NSLOT:
                seq_ranks.append(base + i)
    # core c, slot k gets block order[8*seq_ranks[k] + c].  Slots with
    # more than 128 hits are split into independent virtual slots so
    # every psO accumulation is a single start+stop matmul.
    raw_L = [max(1, int(Ls[order[8 * seq_ranks[k]]])) for k in range(NSLOT)]
    L_slots, vmap = [], []
    for k, L in enumerate(raw_L):
        L_slots.append(min(128, L))
        vmap.append((k, 0))
        if L > 128:
            L_slots.append(L - 128)
            vmap.append((k, 128))
    VN = len(L_slots)
    cslots, grp_of, gg_of, gsz_of, gsizes, Ltot = _pack_units(L_slots)
    units = [u for cs in cslots for u in cs]
    NU = len(units)

    # --- shared RHS: block-local features (hi/lo pairs) + one-hot rows ---
    lx = np.arange(GPTS) // (BY * BZ)
    ly = (np.arange(GPTS) // BZ) % BY
    lz = np.arange(GPTS) % BZ
    xi = ((lx - (BX - 1) / 2.0) * GRID).astype(np.float32)
    yi = ((ly - (BY - 1) / 2.0) * GRID).astype(np.float32)
    zi = ((lz - (BZ - 1) / 2.0) * GRID).astype(np.float32)
    feat10 = np.stack([xi * xi, yi * yi, zi * zi, xi * yi, yi * zi, xi * zi,
                       xi, yi, zi, np.ones(GPTS, np.float32)])
    fhi, flo = _bf16_split(feat10)
    rhs = np.zeros((KF, GPTS), np.float32)
    rhs[0:10] = fhi
    rhs[10:20] = flo
    rhs[20:30] = fhi
    rhs[30 + lx, np.arange(GPTS)] = 1.0
    rhs[30 + BX + ly, np.arange(GPTS)] = 1.0
    rhs[30 + BX + BY + lz, np.arange(GPTS)] = 1.0
    rhs = rhs.astype(ml_dtypes.bfloat16)

    # --- per-core COEF / SEMP ---
    in_maps = []
    perm_blocks = []       # per core: slot -> block id
    # per-slot units (in processing order they appear)
    units_of_slot = {}
    for u in units:
        units_of_slot.setdefault(u[0], []).append(u)
    for ci in range(NCORES):
        coef_m = np.zeros((KF, Ltot), np.float32)
        semp = np.zeros((128, NU * C), np.float16)
        my_blocks = []
        uindex = {}
        for ui, u in enumerate(units):
            uindex[(u[0], u[1])] = ui
        for g in range(VN):
            k, vs0 = vmap[g]
            bid = int(order[8 * seq_ranks[k] + ci])
            my_blocks.append((bid, vs0))
            hit = hits[bid]
            nh = len(hit)
            if nh == 0 or vs0 >= nh:
                continue
            x0, y0, z0 = blocks[bid]
            cx = (x0 + (BX - 1) / 2.0 + 0.5) * GRID
            cy = (y0 + (BY - 1) / 2.0 + 0.5) * GRID
            cz = (z0 + (BZ - 1) / 2.0 + 0.5) * GRID
            dmu = mu[hit] - np.array([cx, cy, cz])      # [nh, 3]
            Ah = A[hit]                                  # [nh, 3, 3]
            Amu = np.einsum('pij,pj->pi', Ah, dmu)
            muAmu = np.einsum('pi,pi->p', dmu, Amu)
            c10 = np.stack([
                -0.5 * Ah[:, 0, 0], -0.5 * Ah[:, 1, 1], -0.5 * Ah[:, 2, 2],
                -Ah[:, 0, 1], -Ah[:, 1, 2], -Ah[:, 0, 2],
                Amu[:, 0], Amu[:, 1], Amu[:, 2],
                -0.5 * muAmu + logop[hit]]).astype(np.float32)
            chi, clo = _bf16_split(c10)
            in_x = (np.abs(x0 + np.arange(BX)[None, :] - mx[hit, None])
                    <= radii[hit, None])
            in_y = (np.abs(y0 + np.arange(BY)[None, :] - my[hit, None])
                    <= radii[hit, None])
            in_z = (np.abs(z0 + np.arange(BZ)[None, :] - mz[hit, None])
                    <= radii[hit, None])
            px = np.where(in_x, 0.0, PEN).T
            py = np.where(in_y, 0.0, PEN).T
            pz = np.where(in_z, 0.0, PEN).T
            semh = sem[hit].astype(np.float16)
            for (gg_, off, Mt, po, first, last, s0, pd) in units_of_slot[g]:
                a, b2 = vs0 + s0, min(vs0 + s0 + Mt, nh)
                if a >= nh:
                    continue
                w_ = b2 - a
                coef_m[0:10, off:off + w_] = chi[:, a:b2]
                coef_m[10:20, off:off + w_] = chi[:, a:b2]
                coef_m[20:30, off:off + w_] = clo[:, a:b2]
                coef_m[30:30 + BX, off:off + w_] = px[:, a:b2]
                coef_m[30 + BX:30 + BX + BY, off:off + w_] = py[:, a:b2]
                coef_m[30 + BX + BY:, off:off + w_] = pz[:, a:b2]
                ui = uindex[(gg_, off)]
                semp[po:po + w_, ui * C:(ui + 1) * C] = semh[a:b2]
        perm_blocks.append(my_blocks)
        coefx = np.concatenate([np.asarray(rhs, np.float32),
                                coef_m], axis=1).astype(ml_dtypes.bfloat16)
        in_maps.append({"COEF": coefx, "SEMP": semp})
    return in_maps, L_slots, (perm_blocks, grp_of, gg_of)


def kernel(**inputs):
    in_maps, L_slots, (perm_blocks, grp_of, gg_of) = _host_prep(**inputs)
    nc = _get_nc(L_slots)
    run = _get_runner(nc)
    results, _, _ = run(in_maps)
    out = np.empty((N, C), np.float32)
    lx = np.arange(GPTS) // (BY * BZ)
    ly = (np.arange(GPTS) // BZ) % BY
    lz = np.arange(GPTS) % BZ
    for ci in range(NCORES):
        o = results[ci]["OUT"].astype(np.float32)
        for g in range(len(perm_blocks[ci])):
            bid, vs0 = perm_blocks[ci][g]
            x0 = (bid // (NBY * NBZ)) * BX
            y0 = ((bid // NBZ) % NBY) * BY
            z0 = (bid % NBZ) * BZ
            grp, gg = grp_of[g], gg_of[g]
            gi, s = gg // PERBANK, gg % PERBANK
            col0 = grp * OCOLS + s * GPTS
            blk = o[gi * 32:gi * 32 + C, col0:col0 + GPTS]   # [C, GPTS]
            gidx = ((x0 + lx) * W + (y0 + ly)) * D + (z0 + lz)
            if vs0 == 0:
                out[gidx] = blk.T
            else:
                out[gidx] += blk.T
    return out



# revision 20
# speedup vs baseline: 1.0548x; 1.0216x over previous
# Trainium2 Bass kernel for nn_LocalAggregator (Gaussian -> voxel-grid semantic
# compositing).
#
# Strategy: the voxel grid (60,60,36) is tiled into 1080 3-D blocks of
# (4,5,6) = 120 voxels.  Blocks are dealt to the 8 cores by sorted hit-count
# (rank 8k+c -> core c, slot k) so every core sees a near-identical load
# profile; the host un-permutes the output afterwards.  For each block the
# host builds the exact list of Gaussians whose integer box overlaps the
# block in all three dims.  In block-local coordinates the feature matrix
# (quadratic monomials + one-hot rows for the x/y/z box tests) is the SAME
# for every block, so a single small RHS is shared by all matmuls; all
# per-(block,Gaussian) data lives in the COEF matrix.  The E matmul runs in
# bf16 with a 3-way hi/lo product split (rows [hi,hi,lo] x features
# [fhi,flo,fhi] drop only the lo*lo term, ~1e-5 relative), Exp runs on the
# scalar engine over 12 units (3 PSUM banks x 4 blocks each) at a time, and
# the semantic reduction is an fp16 matmul accumulated into PSUM banks packed
# 12 blocks deep (3 partition stripes x 4 column slots) so one DVE copy +
# one Pool-engine DMA drains 12 blocks at once.  Dummy matmuls at t=0 ramp
# the PE clock to max p-state and a dummy activation preloads the Exp table.
import numpy as np
import ml_dtypes

H, W, D = 60, 60, 36
GRID = 0.08
SCALE_MULT = 3.0
P = 2048
C = 13
N = H * W * D                  # 129600
NCORES = 8
BX, BY, BZ = 4, 5, 6           # block shape
NBX, NBY, NBZ = H // BX, W // BY, D // BZ
NB = NBX * NBY * NBZ           # 1080 blocks total
GPTS = BX * BY * BZ            # 120 points per block
NSLOT = NB // NCORES           # 135 blocks (slots) per core
NPC = NSLOT * GPTS             # 16200 points per core
KF = 30 + BX + BY + BZ         # 45 feature rows (3x10 split products + onehots)
PEN = -2000.0                  # box-miss penalty (exp() == 0 in fp32)
PERBANK = 4                    # 120-col units per 512-f32 PSUM bank
NBANK = 2                      # banks per psE tile / act instruction
UPT = PERBANK * NBANK          # 8 cslots per psE tile
OGRP = 12                      # blocks per psO bank (3 stripes x 4 col slots)
OROWS = 77                     # 2*32+13 live partitions per output group
OCOLS = PERBANK * GPTS         # 480 cols per output group
NGRP = (NSLOT + OGRP - 1) // OGRP   # output groups per core (last partial)

_NC_CACHE: dict = {}
_JIT_CACHE: dict = {}


def _pack_units(L_slots):
    """Bin-pack units into 128-partition column-slots.

    All E matmuls share one RHS, so any units whose COEF columns are laid
    out adjacently can share a single E matmul (M = sum Mt <= 128) and one
    activation column range.  The per-block semantic matmul then contracts
    K = bin rows from base partition 0; zero semantics rows mask the other
    blocks in the bin.  Returns (cslots, grp_of, gg_of, gsz_of, Ltot) with
    cslots a list of bins, each a list of
    (slot, coef_off, Mt, row_off, first_in_slot, last_in_slot, s0).
    psO groups follow the slot completion order; bins are ordered to keep
    the per-activation-tile unit count flat.
    """
    raw = []
    for g, L in enumerate(L_slots):
        L = int(L)
        for s in range(0, L, 128):
            raw.append((g, min(128, L - s), s))       # (slot, Mt, s0)
    # psO groups in completion order: OGRP-slot groups, plus a tiny final
    # group so the very last drain's copy+DMA chain is short.  Packing
    # regions = consecutive group PAIRS (psO bufs = 2), so a region's bins
    # never reference a psO tile that hasn't been freed by an earlier drain.
    VN_ = len(L_slots)
    k, rem1 = divmod(VN_, OGRP)
    if k == 0:
        gsizes = [rem1]
    elif rem1:
        gsizes = [OGRP] * k + [rem1]
    else:
        gsizes = [OGRP] * (k - 1) + [OGRP - 4, 4]
    # packing regions: pairs of groups, except the final two groups pack
    # SEQUENTIALLY (all of group NG-2's units strictly before group NG-1's)
    # so only the small final group's drain sits on the tail critical path
    regions = []          # (n_slots, sequential_subphases)
    i = 0
    while i < len(gsizes):
        if i + 2 >= len(gsizes):
            regions.append((sum(gsizes[i:]), gsizes[i:]))
            break
        regions.append((gsizes[i] + gsizes[i + 1],
                        [gsizes[i], gsizes[i + 1]]))
        i += 2
    seq = []
    r0 = 0
    for rs, phases in regions:
        for ps in (phases or [rs]):
            idxs = [i for i in range(len(raw))
                    if r0 <= raw[i][0] < r0 + ps]
            idxs.sort(key=lambda i: -raw[i][1])
            bins = []
            for i in idxs:
                Mt = raw[i][1]
                for b in bins:
                    if b[0] + Mt <= 128:
                        b[1].append(i)
                        b[0] += Mt
                        break
                else:
                    bins.append([Mt, [i]])
            seq.extend(bins)
            r0 += ps
    # coef offsets in processing order + row offsets within each bin
    cslots = []
    cum = 0
    placed = {}
    for b in seq:
        out = []
        ro = 0
        for i in b[1]:
            g, Mt, s0 = raw[i]
            out.append([g, cum, Mt, ro, False, False, s0])
            placed[i] = (len(cslots), len(out) - 1)
            cum += Mt
            ro += Mt
        cslots.append(out)
    Ltot = cum
    # first/last flags per slot by processing (cslot) order
    by_slot = {}
    for ci, cs in enumerate(cslots):
        for ji, u in enumerate(cs):
            by_slot.setdefault(u[0], []).append((ci, ji))
    for g, lst in by_slot.items():
        lst.sort()
        ci, ji = lst[0]
        cslots[ci][ji][4] = True
        ci, ji = lst[-1]
        cslots[ci][ji][5] = True
    cslots = [[tuple(u) + (False,) for u in cs] for cs in cslots]
    # slot completion order -> psO groups
    comp = {}
    for ci, cs in enumerate(cslots):
        for u in cs:
            comp[u[0]] = max(comp.get(u[0], -1), ci)
    order = sorted(range(len(L_slots)), key=lambda g: (comp[g], g))
    grp_of = [0] * len(L_slots)
    gg_of = [0] * len(L_slots)
    gsz_of = [0] * len(L_slots)
    gstart = [0]
    for sz in gsizes:
        gstart.append(gstart[-1] + sz)
    for pos, g in enumerate(order):
        q = 0
        while gstart[q + 1] <= pos:
            q += 1
        grp_of[g] = q
        gg_of[g] = pos - gstart[q]
        gsz_of[g] = gsizes[q]
    return cslots, grp_of, gg_of, gsz_of, gsizes, Ltot


def _build_nc(L_slots):
    import concourse.bacc as bacc
    import concourse.tile as tile
    from concourse import mybir

    L_slots = [int(x) for x in L_slots]
    VN = len(L_slots)
    cslots, grp_of, gg_of, gsz_of, gsizes, Ltot = _pack_units(L_slots)
    NG = len(gsizes)
    units = [u for cs in cslots for u in cs]
    NU = len(units)
    NCS = len(cslots)
    # first tile is a single bank (PERBANK cslots) so the first activation
    # fires as soon as possible after the first COEF chunk lands
    tb = [0, min(PERBANK, NCS)]
    while tb[-1] < NCS:
        tb.append(min(tb[-1] + UPT, NCS))
    tiles_u = [(tb[i], cslots[tb[i]:tb[i + 1]]) for i in range(len(tb) - 1)]

    nc = bacc.Bacc("TRN2", target_bir_lowering=False, debug=False,
                   num_devices=NCORES)
    f32 = mybir.dt.float32
    bf16 = mybir.dt.bfloat16
    f16 = mybir.dt.float16
    COEF = nc.dram_tensor("COEF", [KF, GPTS + Ltot], bf16,
                          kind="ExternalInput")
    SEMP = nc.dram_tensor("SEMP", [128, NU * C], f16, kind="ExternalInput")
    OUT = nc.dram_tensor("OUT", [OROWS, NG * OCOLS], f16,
                         kind="ExternalOutput")

    usem = {}
    for ui_, u_ in enumerate(units):
        usem[(u_[0], u_[1])] = ui_

    # drain each psO group right after its last-emitted unit
    last_pos = {}
    for ci_, cs_ in enumerate(cslots):
        for ji_, u_ in enumerate(cs_):
            last_pos[grp_of[u_[0]]] = (ci_, ji_)
    drain_at = {}
    for grp_, pos_ in last_pos.items():
        drain_at.setdefault(pos_, []).append(grp_)

    # coef-chunk boundaries at cslot granularity (processing order):
    # chunk 0 = first tile (HWDGE, lands first), chunk 1 = next ~2 tiles
    # (SWDGE on the idle Pool engine, generated in parallel), chunks 2/3 =
    # the rest split in two (HWDGE) so no tile waits on one huge transfer.
    def _coff(ci):
        return cslots[ci][0][1] if ci < NCS else Ltot
    c1 = min(PERBANK, NCS)
    c2 = min(c1 + 3 * UPT, NCS)
    c3 = min(c2 + 4 * UPT, NCS)
    cb = [0, _coff(c1), _coff(c2), _coff(c3), Ltot]
    # semp split: first half covers units used by the first ~5 tiles
    cs_lens = [len(cs) for cs in cslots]
    u_s0 = sum(cs_lens[:min(5 * UPT, NCS)])
    u_s0 = max(1, min(NU, u_s0))

    with tile.TileContext(nc) as tc:
        with (
            tc.tile_pool(name="big", bufs=1) as big_pool,
            tc.tile_pool(name="w", bufs=6) as w_pool,
            tc.tile_pool(name="og", bufs=6) as og_pool,
            tc.tile_pool(name="psE", bufs=3, space="PSUM") as pse_pool,
            tc.tile_pool(name="psO", bufs=2, space="PSUM") as pso_pool,
        ):
            coefx_b = big_pool.tile([KF, GPTS + Ltot], bf16)
            rhs_b = coefx_b[:, 0:GPTS]
            coef_b = coefx_b[:, GPTS:]
            semp_b = big_pool.tile([128, NU * C], f16)
            scr_b = big_pool.tile([1, 512], bf16)
            scr_o = big_pool.tile([1, 8], f16)

            # --- warmup: preload the Exp activation table while inputs load
            nc.tensor.write(scr_b[0:1, 0:1],
                            np.zeros(1, ml_dtypes.bfloat16).tobytes())
            nc.scalar.activation(scr_o[0:1, 0:1], scr_b[0:1, 0:1],
                                 mybir.ActivationFunctionType.Exp)

            # --- input loads: chunk0 on SP HWDGE (first to land), tiles 1-2
            # + late semp on Pool SWDGE (parallel generation), rest on SP.
            nc.sync.dma_start(coefx_b[:, 0:GPTS + cb[1]],
                              COEF[:, 0:GPTS + cb[1]])
            if cb[1] < cb[2]:
                nc.gpsimd.dma_start(coef_b[:, cb[1]:cb[2]],
                                    COEF[:, GPTS + cb[1]:GPTS + cb[2]])
            nc.sync.dma_start(semp_b[:, 0:u_s0 * C], SEMP[:, 0:u_s0 * C])
            if u_s0 < NU:
                nc.gpsimd.dma_start(semp_b[:, u_s0 * C:], SEMP[:, u_s0 * C:])
            if cb[2] < cb[3]:
                nc.sync.dma_start(coef_b[:, cb[2]:cb[3]],
                                  COEF[:, GPTS + cb[2]:GPTS + cb[3]])
            if cb[3] < Ltot:
                nc.sync.dma_start(coef_b[:, cb[3]:cb[4]],
                                  COEF[:, GPTS + cb[3]:GPTS + cb[4]])

            pso_t = {}    # group -> psO tile
            w_ts, psEs = {}, {}

            def emit_E(t):
                toff, tu = tiles_u[t]
                psE = pse_pool.tile([128, NBANK, 512], f32, name=f"psE{t}",
                                    tag="psE")
                psEs[t] = psE
                for j, cs in enumerate(tu):
                    b, s = j // PERBANK, j % PERBANK
                    off0 = cs[0][1]
                    rows = sum(u[2] for u in cs)
                    nc.tensor.matmul(
                        psE[0:rows, b:b + 1, s * GPTS:(s + 1) * GPTS],
                        coef_b[:, off0:off0 + rows], rhs_b[:],
                        start=True, stop=True, skip_group_check=True)

            def emit_act(t):
                toff, tu = tiles_u[t]
                psE = psEs[t]
                w_t = w_pool.tile([128, NBANK, OCOLS], f16, name=f"w{t}",
                                  tag="w")
                w_ts[t] = w_t
                mtmax = max(sum(u[2] for u in cs) for cs in tu)
                nb_full, rem = divmod(len(tu), PERBANK)
                if nb_full:
                    nc.scalar.activation(
                        w_t[0:mtmax, 0:nb_full, :],
                        psE[0:mtmax, 0:nb_full, 0:OCOLS],
                        mybir.ActivationFunctionType.Exp)
                if rem:
                    nc.scalar.activation(
                        w_t[0:mtmax, nb_full:nb_full + 1, 0:rem * GPTS],
                        psE[0:mtmax, nb_full:nb_full + 1, 0:rem * GPTS],
                        mybir.ActivationFunctionType.Exp)

            def emit_out(t):
                toff, tu = tiles_u[t]
                w_t = w_ts[t]
                for j, cs in enumerate(tu):
                  b, js = j // PERBANK, j % PERBANK
                  rows = sum(u[2] for u in cs)
                  for ji, (g, off, Mt, po, first, last, s0, pd) \
                          in enumerate(cs):
                    grp, gg, gsz = grp_of[g], gg_of[g], gsz_of[g]
                    gi, s = gg // PERBANK, gg % PERBANK
                    ui = usem[(g, off)]
                    if grp not in pso_t:
                        pso_t[grp] = pso_pool.tile([OROWS, OCOLS], f32,
                                                   name=f"psO{grp}",
                                                   tag="psO")
                    nc.tensor.matmul(
                        pso_t[grp][gi * 32:gi * 32 + C,
                                   s * GPTS:(s + 1) * GPTS],
                        semp_b[0:rows, ui * C:(ui + 1) * C],
                        w_t[0:rows, b:b + 1,
                            js * GPTS:(js + 1) * GPTS],
                        start=first, stop=last, skip_group_check=True)
                    for grp_d in drain_at.get((toff + j, ji), []):
                        gsz_d = gsizes[grp_d]
                        top = (gsz_d - 1) // PERBANK   # last stripe index
                        grows = top * 32 + C
                        gcols = OCOLS if top > 0 else gsz_d * GPTS
                        outg = og_pool.tile([OROWS, OCOLS], f16,
                                            name=f"og{grp_d}", tag="og")
                        # final drain on HWDGE; earlier ones alternate
                        # Pool-SWDGE / SP-HWDGE so generation parallelizes
                        dma_eng = (nc.sync if (grp_d == NG - 1
                                               or grp_d % 2 == 1)
                                   else nc.gpsimd)
                        nc.vector.tensor_copy(
                            outg[0:grows, 0:gcols],
                            pso_t[grp_d][0:grows, 0:gcols])
                        dma_eng.dma_start(
                            OUT[0:grows,
                                grp_d * OCOLS:grp_d * OCOLS + gcols],
                            outg[0:grows, 0:gcols])

            emit_E(0)
            for t in range(len(tiles_u)):
                emit_act(t)
                if t + 1 < len(tiles_u):
                    emit_E(t + 1)
                emit_out(t)
    nc.compile()
    return nc


def _get_nc(L_slots):
    key = tuple(int(x) for x in L_slots)
    if key not in _NC_CACHE:
        _NC_CACHE[key] = _build_nc(L_slots)
    return _NC_CACHE[key]


def _get_runner(nc):
    """Cached shard_map-jitted executor for one Bass program (axon/PJRT path).

    Mirrors concourse.bass2jax.run_bass_via_pjrt but keeps the jitted callable
    so repeated runs don't rebuild/recompile."""
    if id(nc) in _JIT_CACHE:
        return _JIT_CACHE[id(nc)]
    import jax
    from concourse import bass2jax, mybir
    from jax.experimental.shard_map import shard_map
    from jax.sharding import Mesh, PartitionSpec

    bass2jax.install_neuronx_cc_hook()
    partition_name = (nc.partition_id_tensor.name
                      if nc.partition_id_tensor else None)
    in_names, out_names, out_avals, zero_outs = [], [], [], []
    for alloc in nc.m.functions[0].allocations:
        if not isinstance(alloc, mybir.MemoryLocationSet):
            continue
        name = alloc.memorylocations[0].name
        if alloc.kind == "ExternalInput":
            if name == partition_name:
                continue
            in_names.append(name)
        elif alloc.kind == "ExternalOutput":
            shape = tuple(alloc.tensor_shape)
            dtype = mybir.dt.np(alloc.dtype)
            out_names.append(name)
            out_avals.append(jax.core.ShapedArray(shape, dtype))
            zero_outs.append(np.zeros(shape, dtype))
    n_params = len(in_names)
    all_in_names = in_names + out_names
    if partition_name is not None:
        all_in_names = all_in_names + [partition_name]

    def _body(*args):
        operands = list(args)
        if partition_name is not None:
            operands.append(bass2jax.partition_id_tensor())
        outs = bass2jax._bass_exec_p.bind(
            *operands,
            out_avals=tuple(out_avals),
            in_names=tuple(all_in_names),
            out_names=tuple(out_names),
            lowering_input_output_aliases=(),
            sim_require_finite=True,
            sim_require_nnan=True,
            nc=nc,
        )
        return tuple(outs)

    devices = jax.devices()[:NCORES]
    mesh = Mesh(np.asarray(devices), ("core",))
    donate = tuple(range(n_params, n_params + len(out_names)))
    sharded = jax.jit(
        shard_map(_body, mesh=mesh,
                  in_specs=(PartitionSpec("core"),) * (n_params + len(out_names)),
                  out_specs=(PartitionSpec("core"),) * len(out_names),
                  check_rep=False),
        donate_argnums=donate, keep_unused=True)

    def run(in_maps, rounds=1):
        concat_in = [np.concatenate([np.asarray(m[nm]) for m in in_maps], axis=0)
                     for nm in in_names]
        outs = None
        for _ in range(rounds):
            zo = [np.concatenate([z] * NCORES, axis=0) for z in zero_outs]
            outs = sharded(*concat_in, *zo)
        outs = [np.asarray(o) for o in outs]
        results = []
        for ci in range(NCORES):
            d = {}
            for oi, nm in enumerate(out_names):
                per = outs[oi].shape[0] // NCORES
                d[nm] = outs[oi][ci * per:(ci + 1) * per]
            results.append(d)
        return results, sharded, (concat_in, zero_outs, in_names, out_names)

    sharded_nd = jax.jit(
        shard_map(_body, mesh=mesh,
                  in_specs=(PartitionSpec("core"),) * (n_params + len(out_names)),
                  out_specs=(PartitionSpec("core"),) * len(out_names),
                  check_rep=False),
        keep_unused=True)

    def timeit(in_maps, iters=30):
        import time as _time
        from jax.sharding import NamedSharding
        sh = NamedSharding(mesh, PartitionSpec("core"))
        concat_in = [np.concatenate([np.asarray(m[nm]) for m in in_maps], axis=0)
                     for nm in in_names]
        zo = [np.concatenate([z] * NCORES, axis=0) for z in zero_outs]
        args = [jax.device_put(a, sh) for a in concat_in + zo]
        outs = sharded_nd(*args)
        jax.block_until_ready(outs)
        t0 = _time.time()
        for _ in range(iters):
            outs = sharded_nd(*args)
        jax.block_until_ready(outs)
        return (_time.time() - t0) / iters

    run.timeit = timeit
    _JIT_CACHE[id(nc)] = run
    return run


def _bf16_split(x):
    hi = x.astype(ml_dtypes.bfloat16).astype(np.float32)
    return hi, (x - hi).astype(np.float32)


def _host_prep(pts, means3D, opacities, semantics, scales, cov3D, origin_use):
    pts = np.asarray(pts, np.float32).reshape(N, 3)
    mu32 = np.asarray(means3D, np.float32).reshape(P, 3)
    op = np.asarray(opacities, np.float64).reshape(P)
    sem = np.asarray(semantics, np.float32).reshape(P, C)
    sc32 = np.asarray(scales, np.float32).reshape(P, 3)
    cov = np.asarray(cov3D, np.float64).reshape(P, 3, 3)
    org32 = np.asarray(origin_use, np.float32).reshape(3)

    # --- integer binning, replicated in fp32 exactly like the reference ---
    radii = np.ceil(sc32.max(-1) * np.float32(SCALE_MULT) / np.float32(GRID)
                    ).astype(np.int32).astype(np.int64)
    m_int = ((mu32 - org32) / np.float32(GRID)).astype(np.int32).astype(np.int64)
    p_int = ((pts - org32) / np.float32(GRID)).astype(np.int32).astype(np.int64)

    # structured-input check: points must be the (i, j, k) voxel-center grid
    idx = np.arange(N)
    kk = idx % D
    col = idx // D
    jj = col % W
    ii = col // W
    grid_int = np.stack([ii, jj, kk], axis=-1)
    if not np.array_equal(p_int, grid_int):
        raise RuntimeError("kernel: unstructured pts not supported by fast path")

    # --- per-Gaussian inverse covariance (float64) ---
    a, b, c_, d, e, f = (cov[:, 0, 0], cov[:, 1, 1], cov[:, 2, 2],
                         cov[:, 0, 1], cov[:, 1, 2], cov[:, 0, 2])
    det = a * (b * c_ - e * e) - d * (d * c_ - e * f) + f * (d * e - b * f)
    ixx = (b * c_ - e * e) / det
    iyy = (a * c_ - f * f) / det
    izz = (a * b - d * d) / det
    ixy = (e * f - d * c_) / det
    iyz = (d * f - a * e) / det
    ixz = (d * e - b * f) / det
    A = np.empty((P, 3, 3))
    A[:, 0, 0], A[:, 1, 1], A[:, 2, 2] = ixx, iyy, izz
    A[:, 0, 1] = A[:, 1, 0] = ixy
    A[:, 1, 2] = A[:, 2, 1] = iyz
    A[:, 0, 2] = A[:, 2, 0] = ixz
    mu = mu32.astype(np.float64)
    logop = np.log(op)

    # --- blocks: bounds, hit lists, load-balanced assignment ---
    mx, my, mz = m_int[:, 0], m_int[:, 1], m_int[:, 2]
    blocks = []            # (x0, y0, z0)
    hits = []
    for bxi in range(NBX):
        for byi in range(NBY):
            for bzi in range(NBZ):
                x0, y0, z0 = bxi * BX, byi * BY, bzi * BZ
                hit = np.where(
                    (mx + radii >= x0) & (mx - radii <= x0 + BX - 1) &
                    (my + radii >= y0) & (my - radii <= y0 + BY - 1) &
                    (mz + radii >= z0) & (mz - radii <= z0 + BZ - 1))[0]
                blocks.append((x0, y0, z0))
                hits.append(hit)
    Ls = np.array([len(h) for h in hits])
    order = np.argsort(-Ls, kind="stable")     # rank r -> block id
    # slot k <- rank seq_ranks[k]: deal ranks from thirds so every
    # region of slots mixes large/mid/small hit lists
    t3 = (NSLOT + 2) // 3
    seq_ranks = []
    for i in range(t3):
        for base in (0, t3, 2 * t3):
            if base + i < NSLOT:
                seq_ranks.append(base + i)
    # core c, slot k gets block order[8*seq_ranks[k] + c].  Slots with
    # more than 128 hits are split into independent virtual slots so
    # every psO accumulation is a single start+stop matmul.
    raw_L = [max(1, int(Ls[order[8 * seq_ranks[k]]])) for k in range(NSLOT)]
    L_slots, vmap = [], []
    for k, L in enumerate(raw_L):
        L_slots.append(min(128, L))
        vmap.append((k, 0))
        if L > 128:
            L_slots.append(L - 128)
            vmap.append((k, 128))
    VN = len(L_slots)
    cslots, grp_of, gg_of, gsz_of, gsizes, Ltot = _pack_units(L_slots)
    units = [u for cs in cslots for u in cs]
    NU = len(units)

    # --- shared RHS: block-local features (hi/lo pairs) + one-hot rows ---
    lx = np.arange(GPTS) // (BY * BZ)
    ly = (np.arange(GPTS) // BZ) % BY
    lz = np.arange(GPTS) % BZ
    xi = ((lx - (BX - 1) / 2.0) * GRID).astype(np.float32)
    yi = ((ly - (BY - 1) / 2.0) * GRID).astype(np.float32)
    zi = ((lz - (BZ - 1) / 2.0) * GRID).astype(np.float32)
    feat10 = np.stack([xi * xi, yi * yi, zi * zi, xi * yi, yi * zi, xi * zi,
                       xi, yi, zi, np.ones(GPTS, np.float32)])
    fhi, flo = _bf16_split(feat10)
    rhs = np.zeros((KF, GPTS), np.float32)
    rhs[0:10] = fhi
    rhs[10:20] = flo
    rhs[20:30] = fhi
    rhs[30 + lx, np.arange(GPTS)] = 1.0
    rhs[30 + BX + ly, np.arange(GPTS)] = 1.0
    rhs[30 + BX + BY + lz, np.arange(GPTS)] = 1.0
    rhs = rhs.astype(ml_dtypes.bfloat16)

    # --- per-core COEF / SEMP ---
    in_maps = []
    perm_blocks = []       # per core: slot -> block id
    # per-slot units (in processing order they appear)
    units_of_slot = {}
    for u in units:
        units_of_slot.setdefault(u[0], []).append(u)
    for ci in range(NCORES):
        coef_m = np.zeros((KF, Ltot), np.float32)
        semp = np.zeros((128, NU * C), np.float16)
        my_blocks = []
        uindex = {}
        for ui, u in enumerate(units):
            uindex[(u[0], u[1])] = ui
        for g in range(VN):
            k, vs0 = vmap[g]
            bid = int(order[8 * seq_ranks[k] + ci])
            my_blocks.append((bid, vs0))
            hit = hits[bid]
            nh = len(hit)
            if nh == 0 or vs0 >= nh:
                continue
            x0, y0, z0 = blocks[bid]
            cx = (x0 + (BX - 1) / 2.0 + 0.5) * GRID
            cy = (y0 + (BY - 1) / 2.0 + 0.5) * GRID
            cz = (z0 + (BZ - 1) / 2.0 + 0.5) * GRID
            dmu = mu[hit] - np.array([cx, cy, cz])      # [nh, 3]
            Ah = A[hit]                                  # [nh, 3, 3]
            Amu = np.einsum('pij,pj->pi', Ah, dmu)
            muAmu = np.einsum('pi,pi->p', dmu, Amu)
            c10 = np.stack([
                -0.5 * Ah[:, 0, 0], -0.5 * Ah[:, 1, 1], -0.5 * Ah[:, 2, 2],
                -Ah[:, 0, 1], -Ah[:, 1, 2], -Ah[:, 0, 2],
                Amu[:, 0], Amu[:, 1], Amu[:, 2],
                -0.5 * muAmu + logop[hit]]).astype(np.float32)
            chi, clo = _bf16_split(c10)
            in_x = (np.abs(x0 + np.arange(BX)[None, :] - mx[hit, None])
                    <= radii[hit, None])
            in_y = (np.abs(y0 + np.arange(BY)[None, :] - my[hit, None])
                    <= radii[hit, None])
            in_z = (np.abs(z0 + np.arange(BZ)[None, :] - mz[hit, None])
                    <= radii[hit, None])
            px = np.where(in_x, 0.0, PEN).T
            py = np.where(in_y, 0.0, PEN).T
            pz = np.where(in_z, 0.0, PEN).T
            semh = sem[hit].astype(np.float16)
            for (gg_, off, Mt, po, first, last, s0, pd) in units_of_slot[g]:
                a, b2 = vs0 + s0, min(vs0 + s0 + Mt, nh)
                if a >= nh:
                    continue
                w_ = b2 - a
                coef_m[0:10, off:off + w_] = chi[:, a:b2]
                coef_m[10:20, off:off + w_] = chi[:, a:b2]
                coef_m[20:30, off:off + w_] = clo[:, a:b2]
                coef_m[30:30 + BX, off:off + w_] = px[:, a:b2]
                coef_m[30 + BX:30 + BX + BY, off:off + w_] = py[:, a:b2]
                coef_m[30 + BX + BY:, off:off + w_] = pz[:, a:b2]
                ui = uindex[(gg_, off)]
                semp[po:po + w_, ui * C:(ui + 1) * C] = semh[a:b2]
        perm_blocks.append(my_blocks)
        coefx = np.concatenate([np.asarray(rhs, np.float32),
                                coef_m], axis=1).astype(ml_dtypes.bfloat16)
        in_maps.append({"COEF": coefx, "SEMP": semp})
    return in_maps, L_slots, (perm_blocks, grp_of, gg_of)


def kernel(**inputs):
    in_maps, L_slots, (perm_blocks, grp_of, gg_of) = _host_prep(**inputs)
    nc = _get_nc(L_slots)
    run = _get_runner(nc)
    results, _, _ = run(in_maps)
    out = np.empty((N, C), np.float32)
    lx = np.arange(GPTS) // (BY * BZ)
    ly = (np.arange(GPTS) // BZ) % BY
    lz = np.arange(GPTS) % BZ
    for ci in range(NCORES):
        o = results[ci]["OUT"].astype(np.float32)
        for g in range(len(perm_blocks[ci])):
            bid, vs0 = perm_blocks[ci][g]
            x0 = (bid // (NBY * NBZ)) * BX
            y0 = ((bid // NBZ) % NBY) * BY
            z0 = (bid % NBZ) * BZ
            grp, gg = grp_of[g], gg_of[g]
            gi, s = gg // PERBANK, gg % PERBANK
            col0 = grp * OCOLS + s * GPTS
            blk = o[gi * 32:gi * 32 + C, col0:col0 + GPTS]   # [C, GPTS]
            gidx = ((x0 + lx) * W + (y0 + ly)) * D + (z0 + lz)
            if vs0 == 0:
                out[gidx] = blk.T
            else:
                out[gidx] += blk.T
    return out

